# revision 35
# baseline (speedup 1.0000x reference)
"""Trainium2 Bass kernel for nn_EnhancedReflectiveCognitiveGraph (GNN edge-softmax attention).

Math (see reference):
  q/k/v = x @ W{q,k,v}.T + b ; per-edge scores s_e = <q[src_e], k[dest_e]>_head / 4
  softmax over edges sharing src (max-subtraction skipped: scores ~ N(0,1) so
  exp never overflows and the weights are mathematically identical)
  agg[dest] += w_e * v[src_e] ; out = agg @ Wo.T + bo

Device strategy (8 cores, node-range sharding, three SPMD launches):
  L1 (proj): each core computes q/k/v (fp16) for its node shard.
  L2 (src phase): core c owns edges with src in its shard, laid out in
      128-edge chunks grouped by 128-node src block.  The k rows for each
      edge slot arrive as a host-prepared per-slot int8 stream (contiguous,
      full DMA bandwidth; per-row quantization scales are applied to the
      reduced scores, not the rows).  q rows are expanded per-edge on-chip
      via PE matmuls against streamed one-hot matrices in [feat x slot]
      orientation; the per-head dot products are then a second PE matmul
      against a tiny constant block-diagonal matrix, so no DVE reduction is
      needed.  exp -> per-src-block segment sums via PE matmuls with
      one-hots -> reciprocal -> per-edge softmax weights w_e (output).
  L3 (dest phase): core c owns edges with dest in its shard.  v rows arrive
      as a per-slot int8 stream; weighted rows (w_e * v) are scatter-added
      into per-dest-block agg via PE matmuls with one-hots, then the output
      projection.  No collectives and no device-side gathers anywhere.
  Host between launches does relayout only: assembling tables from L1/L2
  outputs, per-row int8 packing, per-slot stream/one-hot construction, and
  permutation of edge weights between the src- and dest-groupings.
"""

import math
import ml_dtypes
import numpy as np

import concourse.bacc as bacc
import concourse.mybir as mybir
import concourse.tile as tile
from concourse.bass_utils import run_bass_kernel_spmd

# ---------------------------------------------------------------- constants
N = 50000
E = 600000
F = 128
H = 8
Dh = 16
P = 128
C = 8                     # cores
SH = 6272                 # nodes per core, cores 0-6 (49 blocks); core 7: 6096
NB = 49                   # blocks per shard
G = 8                     # chunks per processing group (psum-sized)
KB = 64                   # chunks per stream DMA tile
SG = 12                   # blocks per recip supergroup in L2
F16 = mybir.dt.float16
F8 = mybir.dt.float8e4
F32 = mybir.dt.float32
I8 = mybir.dt.int8


def shard_base(c):
    return c * SH


def shard_len(c):
    return min(N, (c + 1) * SH) - c * SH


# ---------------------------------------------------------------- host prep
class ChunkMap:
    """Uniform chunk structure shared by all cores for one phase.

    Chunks (128 slots each) are block-major: kb[b] chunks for block b; the
    chunk->block map is identical on every core so one program serves all 8."""

    def __init__(self, kb):
        self.kb = [int(x) for x in kb]
        self.chunks = [b for b in range(NB) for _ in range(self.kb[b])]
        self.nch = len(self.chunks)
        self.nslots = self.nch * P
        self.start = np.concatenate([[0], np.cumsum(self.kb)]).astype(int)


def compute_cmap(key, other=None):
    """Global uniform per-block chunk counts for one phase."""
    kb = np.ones(NB, np.int64)
    for c in range(C):
        base, ln = shard_base(c), shard_len(c)
        m = (key >= base) & (key < base + ln)
        cnt = np.bincount((key[m] - base) // P, minlength=NB)
        kb = np.maximum(kb, (cnt + P - 1) // P)
    return ChunkMap(kb)


def group_list(cmap):
    """Processing groups of up to G chunks, block-aligned: (b, g0, gn, cs, ce)."""
    groups = []
    for b in range(NB):
        cs, ce = int(cmap.start[b]), int(cmap.start[b + 1])
        for g0 in range(cs, ce, G):
            groups.append((b, g0, min(G, ce - g0), cs, ce))
    return groups


def chunk_split(cmap, num, den):
    """Mixed-precision chunk routing: groups with gi % den < num take the
    f16 path (Act-assisted 2x multiply), the rest the int8 path (half DMA).
    Returns (f16 flag per chunk, compact offset per chunk, n8, n16)."""
    f16 = np.zeros(cmap.nch, bool)
    for i, (b, g0, gn, cs, ce) in enumerate(group_list(cmap)):
        if i % den < num:
            f16[g0:g0 + gn] = True
    off = np.zeros(cmap.nch, np.int64)
    n8 = n16 = 0
    for ci in range(cmap.nch):
        if f16[ci]:
            off[ci] = n16
            n16 += 1
        else:
            off[ci] = n8
            n8 += 1
    return f16, off, n8, n16


class CorePlan:
    """Per-core slot contents for one phase.  `key` = node defining the block
    (src for L2, dest for L3); `other` = node whose row the slot consumes."""

    def __init__(self, cmap, core, key, other, edge_ids):
        base = shard_base(core)
        nsl = cmap.nslots
        self.slot_local = np.full(nsl, -1, np.int64)
        self.slot_other = np.zeros(nsl, np.int64)
        self.slot_edge = np.full(nsl, -1, np.int64)
        block = (key - base) // P
        for b in range(NB):
            m = block == b
            cnt = int(m.sum())
            if cnt == 0:
                continue
            assert cnt <= cmap.kb[b] * P
            s0 = int(cmap.start[b]) * P
            self.slot_local[s0:s0 + cnt] = key[m] - base - b * P
            self.slot_other[s0:s0 + cnt] = other[m]
            self.slot_edge[s0:s0 + cnt] = edge_ids[m]
        self.cmap = cmap

    def onehot_stream(self, transposed):
        """[128, nch*128] fp8; chunk c at cols c*128:(c+1)*128.
        transposed=False: S [loc, (c,slot)] ; True: ST/TT [slot, (c,loc)].
        Dummy slots are all-zero columns/rows."""
        cm = self.cmap
        out = np.zeros((P, cm.nch * P), dtype=ml_dtypes.float8_e4m3)
        loc = self.slot_local
        sl_all = np.arange(cm.nslots)
        valid = loc >= 0
        ch = sl_all // P
        row = sl_all % P
        if transposed:
            out[row[valid], ch[valid] * P + loc[valid]] = 1.0
        else:
            out[loc[valid], ch[valid] * P + row[valid]] = 1.0
        return out


def row_quant_int8(a16):
    """Per-row symmetric int8 quantization.  Returns (int8 vals, f32 scales)."""
    a = np.asarray(a16, np.float32)
    am = np.abs(a).max(axis=1)
    s = np.where(am > 0, am / 127.0, 1.0).astype(np.float32)
    q = np.clip(np.round(a / s[:, None]), -127, 127).astype(np.int8)
    return q, s


def head_mask_matrix():
    """[128, 8] fp8 block-diagonal ones: B[f, h] = (f // 16 == h)."""
    B = np.zeros((P, H), dtype=ml_dtypes.float8_e4m3)
    for h in range(H):
        B[h * Dh:(h + 1) * Dh, h] = 1.0
    return B


# ---------------------------------------------------------------- L1: projections
def build_l1():
    nc = bacc.Bacc("TRN2", target_bir_lowering=False, num_devices=C)
    xT = nc.dram_tensor("xT", [P, NB * P], F16, kind="ExternalInput")
    wqkv = nc.dram_tensor("wqkv", [P, 3 * P], F16, kind="ExternalInput")
    bqkv = nc.dram_tensor("bqkv", [1, 3 * P], F16, kind="ExternalInput")
    ones = nc.dram_tensor("ones", [1, P], F16, kind="ExternalInput")
    qkv = nc.dram_tensor("qkv", [P, NB * 3 * P], F16, kind="ExternalOutput")

    with tile.TileContext(nc) as tc:
        with tc.tile_pool(name="const", bufs=1) as cpool, \
             tc.tile_pool(name="stage", bufs=3) as spool, \
             tc.tile_pool(name="psum", bufs=4, space="PSUM") as ppool:
            w_sb = cpool.tile([P, 3 * P], F16, tag="w", name="w_sb")
            nc.sync.dma_start(w_sb[:], wqkv[:])
            b_sb = cpool.tile([1, 3 * P], F16, tag="b", name="b_sb")
            nc.sync.dma_start(b_sb[:], bqkv[:])
            ones_sb = cpool.tile([1, P], F16, tag="ones", name="ones_sb")
            nc.sync.dma_start(ones_sb[:], ones[:])
            xt = cpool.tile([P, NB * P], F16, tag="xT", name="xt")
            for i in range(4):
                a = i * 13 * P
                b = min(NB, (i + 1) * 13) * P
                nc.sync.dma_start(xt[:, a:b], xT[:, a:b])
            DB = 7   # blocks per output DMA
            osb = cpool.tile([P, NB * 3 * P], F16, tag="osb", name="osb")
            for b in range(NB):
                ps = ppool.tile([P, 3 * P], F32, tag="proj", name="ps")
                nc.tensor.matmul(ps[:], lhsT=xt[:, b * P:(b + 1) * P],
                                 rhs=w_sb[:], start=True, stop=False)
                nc.tensor.matmul(ps[:], lhsT=ones_sb[:], rhs=b_sb[:],
                                 start=False, stop=True)
                dstsl = osb[:, b * 3 * P:(b + 1) * 3 * P]
                if b % 2 == 0:
                    nc.vector.tensor_copy(dstsl, ps[:])
                else:
                    nc.scalar.copy(dstsl, ps[:])
                if b % DB == DB - 1 or b == NB - 1:
                    a0 = (b // DB) * DB * 3 * P
                    a1 = (b + 1) * 3 * P
                    nc.sync.dma_start(qkv[:, a0:a1], osb[:, a0:a1])
    nc.compile()
    return nc


# ---------------------------------------------------------------- L2: src phase
L2_SPLIT = (1, 4)   # 1/4 of groups take the f16 k path


def build_l2(cmap):
    nch = cmap.nch
    kf16, koff, n8, n16 = chunk_split(cmap, *L2_SPLIT)
    nc = bacc.Bacc("TRN2", target_bir_lowering=False, num_devices=C)
    q_sh = nc.dram_tensor("q_sh", [P, NB * P], F16, kind="ExternalInput")
    kst8 = nc.dram_tensor("kst8", [P, max(n8, 1) * P], I8, kind="ExternalInput")
    kst16 = nc.dram_tensor("kst16", [P, max(n16, 1) * P], F16,
                           kind="ExternalInput")
    srow = nc.dram_tensor("srow", [P, nch], F16, kind="ExternalInput")
    S_st = nc.dram_tensor("S_st", [P, nch * P], F8, kind="ExternalInput")
    ST_st = nc.dram_tensor("ST_st", [P, nch * P], F8, kind="ExternalInput")
    Bm = nc.dram_tensor("Bm", [P, H], F8, kind="ExternalInput")
    exp_out = nc.dram_tensor("exp_out", [P, nch * H], F16, kind="ExternalOutput")
    rec_out = nc.dram_tensor("rec_out", [P, NB * H], F16, kind="ExternalOutput")

    groups = group_list(cmap)
    ng = len(groups)

    with tile.TileContext(nc) as tc:
        with tc.tile_pool(name="res", bufs=1) as rpool, \
             tc.tile_pool(name="kst", bufs=2) as kpool, \
             tc.tile_pool(name="st", bufs=2) as tpool, \
             tc.tile_pool(name="sst", bufs=2) as spool, \
             tc.tile_pool(name="work", bufs=4) as wpool, \
             tc.tile_pool(name="qpsum", bufs=2, space="PSUM") as qpsum, \
             tc.tile_pool(name="spsum", bufs=2, space="PSUM") as spsum, \
             tc.tile_pool(name="gpsum", bufs=2, space="PSUM") as gpsum:
            B_sb = rpool.tile([P, H], F8, tag="B", name="B_sb")
            nc.sync.dma_start(B_sb[:], Bm[:])
            srow_sb = rpool.tile([P, nch], F16, tag="srow", name="srow_sb")
            nc.sync.dma_start(srow_sb[:], srow[:])
            q_sb = rpool.tile([P, NB * P], F16, tag="q", name="q_sb")
            for i in range(4):
                a = (i * NB // 4) * P
                b = ((i + 1) * NB // 4) * P
                nc.sync.dma_start(q_sb[:, a:b], q_sh[:, a:b])
            exp_sb = rpool.tile([P, nch * H], F16, tag="exp", name="exp_sb")
            seg_sb = rpool.tile([P, NB * H], F32, tag="seg", name="seg_sb")
            rec_sb = rpool.tile([P, NB * H], F16, tag="rec", name="rec_sb")

            k_tiles = {}
            k16_tiles = {}
            t_tiles = {}
            s_tiles = {}

            def stream(tiles, pool, dram, ci, width=P, dt=I8, total=None,
                       off=None):
                cc = ci if off is None else int(off[ci])
                tot = nch if total is None else total
                t0 = cc // KB * KB
                if t0 not in tiles:
                    t = pool.tile([P, KB * width], dt, tag=dram.name,
                                  name=f"strm_{dram.name}_{t0}")
                    n = min(KB, tot - t0) * width
                    nc.sync.dma_start(t[:, :n], dram[:, t0 * width:t0 * width + n])
                    tiles[t0] = t
                return tiles[t0], t0, cc

            # Software-pipelined stages, skewed so PE never queue-stalls on a
            # cross-engine dependency:
            #   A(i):   q expansion (PE) + qk multiply (DVE)
            #   B(i-1): score matmuls (PE)
            #   C(i-2): on last group of a block: dequant (DVE), exp (Act),
            #           segment-sum matmuls (PE), seg copy (Act)
            qkTs = {}
            scps = {}

            def stage_a(i):
                b, g0, gn, cs, ce = groups[i]
                qeT = qpsum.tile([P, G * P], F32, tag="qeT", name="qeT")
                ci = g0
                while ci < g0 + gn:
                    st, t0, _ = stream(s_tiles, spool, S_st, ci, dt=F8)
                    # pieces must not cross 512-col psum bank boundaries
                    cj = min(g0 + gn, t0 + KB, g0 + ((ci - g0) // 4 + 1) * 4)
                    nsl = (cj - ci) * P
                    nc.tensor.matmul(
                        qeT[:, (ci - g0) * P:(ci - g0) * P + nsl],
                        lhsT=q_sb[:, b * P:(b + 1) * P],
                        rhs=st[:, (ci - t0) * P:(ci - t0) * P + nsl],
                        start=True, stop=True)
                    ci = cj
                qkT = wpool.tile([P, G * P], F16, tag="qkT", name="qkT")
                f16p = bool(kf16[g0])
                if f16p:
                    qe16 = wpool.tile([P, G * P], F16, tag="qe16", name="qe16")
                    nc.scalar.copy(qe16[:, :gn * P], qeT[:, :gn * P])
                    in0 = qe16
                else:
                    in0 = qeT
                ci = g0
                while ci < g0 + gn:
                    if f16p:
                        kt, t0, cc = stream(k16_tiles, kpool, kst16, ci,
                                            dt=F16, total=n16, off=koff)
                    else:
                        kt, t0, cc = stream(k_tiles, kpool, kst8, ci,
                                            dt=I8, total=n8, off=koff)
                    cj = min(g0 + gn, ci + (t0 + KB - cc))
                    nsl = (cj - ci) * P
                    nc.vector.tensor_mul(
                        qkT[:, (ci - g0) * P:(ci - g0) * P + nsl],
                        in0[:, (ci - g0) * P:(ci - g0) * P + nsl],
                        kt[:, (cc - t0) * P:(cc - t0) * P + nsl])
                    ci = cj
                qkTs[i] = qkT

            def stage_b(i):
                b, g0, gn, cs, ce = groups[i]
                qkT = qkTs.pop(i)
                if g0 == cs:
                    scps[b] = spsum.tile([P, 16 * H], F32, tag="sc",
                                         name="scp")
                scp = scps[b]
                for ci in range(g0, g0 + gn):
                    nc.tensor.matmul(
                        scp[:, (ci - cs) * H:(ci - cs + 1) * H],
                        lhsT=qkT[:, (ci - g0) * P:(ci - g0 + 1) * P],
                        rhs=B_sb[:], start=True, stop=True)

            def stage_c(i):
                b, g0, gn, cs, ce = groups[i]
                if g0 + gn != ce:
                    return
                nb = ce - cs
                scp = scps.pop(b)
                sc16 = wpool.tile([P, 16 * H], F16, tag="sc16", name="sc16")
                nc.vector.tensor_mul(
                    sc16[:, :nb * H].rearrange("p (c h) -> p c h", h=H),
                    scp[:, :nb * H].rearrange("p (c h) -> p c h", h=H),
                    srow_sb[:, cs:ce][:, :, None]
                    .broadcast_to([P, nb, H]))
                nc.scalar.activation(
                    out=exp_sb[:, cs * H:ce * H],
                    in_=sc16[:, :nb * H],
                    func=mybir.ActivationFunctionType.Exp,
                    scale=1.0)
                segp = gpsum.tile([P, H], F32, tag="seg", name="segp")
                for ci in range(cs, ce):
                    tt, t0, _ = stream(t_tiles, tpool, ST_st, ci, dt=F8)
                    nc.tensor.matmul(
                        segp[:],
                        lhsT=tt[:, (ci - t0) * P:(ci - t0 + 1) * P],
                        rhs=exp_sb[:, ci * H:(ci + 1) * H],
                        start=(ci == cs), stop=(ci == ce - 1))
                nc.scalar.copy(seg_sb[:, b * H:(b + 1) * H], segp[:])

            for i in range(ng + 2):
                if i < ng:
                    stage_a(i)
                if 1 <= i <= ng:
                    stage_b(i - 1)
                if 2 <= i <= ng + 1:
                    stage_c(i - 2)
                    bdone = groups[i - 2][0]
                    if groups[i - 2][1] + groups[i - 2][2] == groups[i - 2][4]:
                        # exp_out slice per ~8 finished blocks
                        if bdone % 8 == 7 or bdone == NB - 1:
                            a = int(cmap.start[bdone // 8 * 8]) * H
                            bcol = int(cmap.start[bdone + 1]) * H
                            nc.sync.dma_start(exp_out[:, a:bcol],
                                              exp_sb[:, a:bcol])

            # reciprocal; empty segments (zero-degree locs, padding) get
            # seg+1 so it stays finite — their one-hot columns are all-zero
            # downstream so the value never contributes.
            seg1 = wpool.tile([P, NB * H], F32, tag="seg1", name="seg1")
            nc.vector.scalar_tensor_tensor(
                out=seg1[:], in0=seg_sb[:], scalar=0.0, in1=seg_sb[:],
                op0=mybir.AluOpType.is_le, op1=mybir.AluOpType.add)
            with nc.allow_low_precision(reason="softmax recip fits f16"):
                nc.vector.reciprocal(rec_sb[:], seg1[:])
            nc.sync.dma_start(rec_out[:], rec_sb[:])
    nc.compile()
    return nc


# ---------------------------------------------------------------- L3: dest phase
L3_SPLIT = (2, 5)   # 2/5 of groups take the f16 v path


def build_l3(cmap):
    nch = cmap.nch
    vf16, voff, n8, n16 = chunk_split(cmap, *L3_SPLIT)
    nc = bacc.Bacc("TRN2", target_bir_lowering=False, num_devices=C)
    vst8 = nc.dram_tensor("vst8", [P, max(n8, 1) * P], I8, kind="ExternalInput")
    vst16 = nc.dram_tensor("vst16", [P, max(n16, 1) * P], F16,
                           kind="ExternalInput")
    exp_in = nc.dram_tensor("exp_in", [P, nch * H], F16, kind="ExternalInput")
    srw = nc.dram_tensor("srw", [P, nch * H], F16, kind="ExternalInput")
    TT_st = nc.dram_tensor("TT_st", [P, nch * P], F8, kind="ExternalInput")
    WoT = nc.dram_tensor("WoT", [P, P], F16, kind="ExternalInput")
    bo_r = nc.dram_tensor("bo_r", [1, P], F16, kind="ExternalInput")
    ones = nc.dram_tensor("ones", [1, P], F16, kind="ExternalInput")
    outT = nc.dram_tensor("outT", [P, NB * P], F16, kind="ExternalOutput")

    with tile.TileContext(nc) as tc:
        with tc.tile_pool(name="res", bufs=1) as rpool, \
             tc.tile_pool(name="vstp", bufs=2) as vpool, \
             tc.tile_pool(name="tt", bufs=2) as tpool, \
             tc.tile_pool(name="work", bufs=3) as wpool, \
             tc.tile_pool(name="apsum", bufs=2, space="PSUM") as apsum, \
             tc.tile_pool(name="opsum", bufs=2, space="PSUM") as opsum:
            wo_sb = rpool.tile([P, P], F16, tag="wo", name="wo_sb")
            nc.sync.dma_start(wo_sb[:], WoT[:])
            bo_sb = rpool.tile([1, P], F16, tag="bo", name="bo_sb")
            nc.sync.dma_start(bo_sb[:], bo_r[:])
            ones_sb = rpool.tile([1, P], F16, tag="ones", name="ones_sb")
            nc.sync.dma_start(ones_sb[:], ones[:])
            osb = rpool.tile([P, NB * P], F16, tag="osb", name="osb")
            w_sb = rpool.tile([P, nch * H], F16, tag="w", name="w_sb")
            sv_sb = rpool.tile([P, nch * H], F16, tag="sv", name="sv_sb")
            wsc = rpool.tile([P, nch * H], F16, tag="wsc", name="wsc")

            v_tiles = {}
            v16_tiles = {}
            t_tiles = {}
            w_spans = set()

            def stream(tiles, pool, dram, ci, dt, total=None, off=None):
                cc = ci if off is None else int(off[ci])
                tot = nch if total is None else total
                t0 = cc // KB * KB
                if t0 not in tiles:
                    t = pool.tile([P, KB * P], dt, tag=dram.name,
                                  name=f"strm_{dram.name}_{t0}")
                    n = min(KB, tot - t0) * P
                    nc.sync.dma_start(t[:, :n], dram[:, t0 * P:t0 * P + n])
                    tiles[t0] = t
                return tiles[t0], t0, cc

            def want_wsc(ci):
                # per-edge weight exp * (rec[src] * v-row-scale): lazily DMA'd
                # and computed (packed f16 2x mult) per KB-chunk span
                t0 = ci // KB * KB
                if t0 not in w_spans:
                    w_spans.add(t0)
                    a = t0 * H
                    bcol = min(nch, t0 + KB) * H
                    nc.sync.dma_start(w_sb[:, a:bcol], exp_in[:, a:bcol])
                    nc.sync.dma_start(sv_sb[:, a:bcol], srw[:, a:bcol])
                    nc.vector.tensor_mul(wsc[:, a:bcol], w_sb[:, a:bcol],
                                         sv_sb[:, a:bcol])

            gi = 0
            DB = 7
            for b in range(NB):
                cs, ce = int(cmap.start[b]), int(cmap.start[b + 1])
                aggp = apsum.tile([P, P], F32, tag="agg", name="aggp")
                for g0 in range(cs, ce, G):
                    gn = min(G, ce - g0)
                    wv = wpool.tile([P, G * P], F16, tag="wv", name="wv")
                    want_wsc(g0)
                    want_wsc(g0 + gn - 1)
                    f16p = bool(vf16[g0])
                    erep = None
                    if f16p:
                        # materialize the broadcast weights on Act so the
                        # multiply runs packed-f16 at DVE 2x
                        erep = wpool.tile([P, G * P], F16, tag="erep",
                                          name="erep")
                        nc.scalar.copy(
                            erep[:, :gn * P]
                            .rearrange("p (c h d) -> p c h d", h=H, d=Dh),
                            wsc[:, g0 * H:(g0 + gn) * H]
                            .rearrange("p (c h) -> p c h", h=H)[:, :, :, None]
                            .broadcast_to([P, gn, H, Dh]))
                    ci = g0
                    while ci < g0 + gn:
                        if f16p:
                            vt, t0, cc = stream(v16_tiles, vpool, vst16, ci,
                                                F16, total=n16, off=voff)
                        else:
                            vt, t0, cc = stream(v_tiles, vpool, vst8, ci,
                                                I8, total=n8, off=voff)
                        cj = min(g0 + gn, ci + (t0 + KB - cc))
                        nn = cj - ci
                        if f16p:
                            nc.vector.tensor_mul(
                                wv[:, (ci - g0) * P:(ci - g0 + nn) * P],
                                vt[:, (cc - t0) * P:(cc - t0 + nn) * P],
                                erep[:, (ci - g0) * P:(ci - g0 + nn) * P])
                        else:
                            nc.vector.tensor_mul(
                                wv[:, (ci - g0) * P:(ci - g0 + nn) * P]
                                .rearrange("p (c h d) -> p c h d", h=H, d=Dh),
                                vt[:, (cc - t0) * P:(cc - t0 + nn) * P]
                                .rearrange("p (c h d) -> p c h d", h=H, d=Dh),
                                wsc[:, ci * H:(ci + nn) * H]
                                .rearrange("p (c h) -> p c h", h=H)
                                [:, :, :, None]
                                .broadcast_to([P, nn, H, Dh]))
                        ci = cj
                    gi += 1
                    for ci in range(g0, g0 + gn):
                        tt, t0, _ = stream(t_tiles, tpool, TT_st, ci, F8)
                        nc.tensor.matmul(
                            aggp[:],
                            lhsT=wv[:, (ci - g0) * P:(ci - g0 + 1) * P],
                            rhs=tt[:, (ci - t0) * P:(ci - t0 + 1) * P],
                            start=(ci == cs), stop=(ci == ce - 1))
                agg16 = wpool.tile([P, P], F16, tag="agg16", name="agg16")
                nc.scalar.copy(agg16[:], aggp[:])
                outp = opsum.tile([P, P], F32, tag="outp", name="outp")
                nc.tensor.matmul(outp[:], lhsT=wo_sb[:], rhs=agg16[:],
                                 start=True, stop=False)
                nc.tensor.matmul(outp[:], lhsT=bo_sb[:], rhs=ones_sb[:],
                                 start=False, stop=True)
                nc.scalar.copy(osb[:, b * P:(b + 1) * P], outp[:])
                if b % DB == DB - 1 or b == NB - 1:
                    a0 = (b // DB) * DB * P
                    a1 = (b + 1) * P
                    nc.sync.dma_start(outT[:, a0:a1], osb[:, a0:a1])
    nc.compile()
    return nc


# ---------------------------------------------------------------- orchestration
def _prep_weights(Wq, bq, Wk, bk, Wv, bv, Wo, bo):
    w16 = {k: np.asarray(v, np.float32).astype(np.float16)
           for k, v in (("Wq", Wq), ("Wk", Wk), ("Wv", Wv), ("Wo", Wo))}
    b16 = {k: np.asarray(v, np.float32).astype(np.float16)
           for k, v in (("bq", bq), ("bk", bk), ("bv", bv), ("bo", bo))}
    return w16, b16


def kernel(node_features, edge_index, Wq, bq, Wk, bk, Wv, bv, Wo, bo):
    node_features = np.asarray(node_features, np.float32)
    edge_index = np.asarray(edge_index)
    src, dst = edge_index[0].astype(np.int64), edge_index[1].astype(np.int64)
    x16 = node_features.astype(np.float16)
    w16, b16 = _prep_weights(Wq, bq, Wk, bk, Wv, bv, Wo, bo)
    ones_row = np.ones((1, P), np.float16)
    cores = list(range(C))
    eids = np.arange(E, dtype=np.int64)

    # ---------------- L1
    nc1 = build_l1()
    in1 = []
    for c in cores:
        base, ln = shard_base(c), shard_len(c)
        xt = np.zeros((P, NB * P), np.float16)
        xt[:, :ln] = x16[base:base + ln].T
        in1.append(dict(
            xT=xt,
            wqkv=np.concatenate([w16["Wq"].T, w16["Wk"].T, w16["Wv"].T],
                                axis=1).copy(),
            bqkv=np.concatenate([b16["bq"], b16["bk"], b16["bv"]])
            .reshape(1, 3 * P), ones=ones_row))
    r1 = run_bass_kernel_spmd(nc1, in1, core_ids=cores)

    q_shs = []
    k_all = np.zeros((N, P), np.float16)
    v_all = np.zeros((N, P), np.float16)
    for c in cores:
        base, ln = shard_base(c), shard_len(c)
        blob = r1.results[c]["qkv"].reshape(P, NB, 3, P)
        q_shs.append(np.ascontiguousarray(blob[:, :, 0, :].reshape(P, NB * P)))
        k_sh = blob[:, :, 1, :].transpose(1, 0, 2).reshape(NB * P, P)
        v_sh = blob[:, :, 2, :].transpose(1, 0, 2).reshape(NB * P, P)
        k_all[base:base + ln] = k_sh[:ln]
        v_all[base:base + ln] = v_sh[:ln]

    k8, krs = row_quant_int8(k_all)
    v8, vrs = row_quant_int8(v_all)

    # ---------------- L2
    cmap2 = compute_cmap(src)
    plans2 = []
    for c in cores:
        base, ln = shard_base(c), shard_len(c)
        m = (src >= base) & (src < base + ln)
        plans2.append(CorePlan(cmap2, c, src[m], dst[m], eids[m]))

    nc2 = build_l2(cmap2)
    Bmat = head_mask_matrix()
    kf16, koff, kn8, kn16 = chunk_split(cmap2, *L2_SPLIT)
    in2 = []
    for c in cores:
        pl = plans2[c]
        nch = cmap2.nch
        oth = pl.slot_other.reshape(nch, P)
        k8_rows = k8[oth[~kf16]]                       # [n8, P, P] int8
        k16_rows = k_all[oth[kf16]]                    # [n16, P, P] f16
        kst8 = k8_rows.transpose(2, 0, 1).reshape(P, -1).copy() \
            if kn8 else np.zeros((P, P), np.int8)
        kst16 = k16_rows.transpose(2, 0, 1).reshape(P, -1).copy() \
            if kn16 else np.zeros((P, P), np.float16)
        valid = (pl.slot_edge >= 0).astype(np.float32)
        scale = np.where(np.repeat(kf16, P), 1.0, krs[pl.slot_other])
        srow_v = (scale * valid * 0.25).astype(np.float16)
        in2.append(dict(
            q_sh=q_shs[c], kst8=kst8, kst16=kst16,
            srow=np.ascontiguousarray(srow_v.reshape(nch, P).T),
            S_st=pl.onehot_stream(False), ST_st=pl.onehot_stream(True),
            Bm=Bmat))
    r2 = run_bass_kernel_spmd(nc2, in2, core_ids=cores)

    exp_edge = np.zeros((E, H), np.float16)
    rec_all = np.zeros((N, H), np.float16)
    for c in cores:
        pl = plans2[c]
        e_flat = r2.results[c]["exp_out"].reshape(P, cmap2.nch, H) \
            .transpose(1, 0, 2).reshape(cmap2.nslots, H)
        real = pl.slot_edge >= 0
        exp_edge[pl.slot_edge[real]] = e_flat[real]
        base, ln = shard_base(c), shard_len(c)
        rec_sh = r2.results[c]["rec_out"].reshape(P, NB, H) \
            .transpose(1, 0, 2).reshape(NB * P, H)
        rec_all[base:base + ln] = rec_sh[:ln]

    # ---------------- L3
    cmap3 = compute_cmap(dst)
    plans3 = []
    for c in cores:
        base, ln = shard_base(c), shard_len(c)
        m = (dst >= base) & (dst < base + ln)
        plans3.append(CorePlan(cmap3, c, dst[m], src[m], eids[m]))

    nc3 = build_l3(cmap3)
    vf16, voff, vn8, vn16 = chunk_split(cmap3, *L3_SPLIT)
    in3 = []
    for c in cores:
        pl = plans3[c]
        nch = cmap3.nch
        oth = pl.slot_other.reshape(nch, P)
        v8_rows = v8[oth[~vf16]]
        v16_rows = v_all[oth[vf16]]
        vst8 = v8_rows.transpose(1, 0, 2).reshape(P, -1).copy() \
            if vn8 else np.zeros((P, P), np.int8)
        vst16 = v16_rows.transpose(1, 0, 2).reshape(P, -1).copy() \
            if vn16 else np.zeros((P, P), np.float16)
        e_slots = np.zeros((cmap3.nslots, H), np.float16)
        real = pl.slot_edge >= 0
        e_slots[real] = exp_edge[pl.slot_edge[real]]
        # combined per-slot scale: softmax denominator recip at the src node
        # times the src v-row int8 scale (1 for f16 chunks, 0 on padding)
        vscale = np.where(np.repeat(vf16, P), 1.0, vrs[pl.slot_other])
        srw_v = (rec_all[pl.slot_other].astype(np.float32) *
                 (vscale * real.astype(np.float32))[:, None]) \
            .astype(np.float16)
        in3.append(dict(
            vst8=vst8, vst16=vst16,
            exp_in=np.ascontiguousarray(
                e_slots.reshape(nch, P, H).transpose(1, 0, 2)
                .reshape(P, nch * H)),
            srw=np.ascontiguousarray(
                srw_v.reshape(nch, P, H).transpose(1, 0, 2)
                .reshape(P, nch * H)),
            TT_st=pl.onehot_stream(True),
            WoT=w16["Wo"].T.copy(),
            bo_r=b16["bo"].reshape(1, P), ones=ones_row))
    r3 = run_bass_kernel_spmd(nc3, in3, core_ids=cores)

    out = np.zeros((N, F), np.float32)
    for c in cores:
        base, ln = shard_base(c), shard_len(c)
        o = r3.results[c]["outT"].reshape(P, NB, P).transpose(1, 2, 0) \
            .reshape(NB * P, P)
        out[base:base + ln] = o[:ln].astype(np.float32)
    return out


# revision 36
# speedup vs baseline: 1.0068x; 1.0068x over previous
"""Trainium2 Bass kernel for nn_EnhancedReflectiveCognitiveGraph (GNN edge-softmax attention).

Math (see reference):
  q/k/v = x @ W{q,k,v}.T + b ; per-edge scores s_e = <q[src_e], k[dest_e]>_head / 4
  softmax over edges sharing src (max-subtraction skipped: scores ~ N(0,1) so
  exp never overflows and the weights are mathematically identical)
  agg[dest] += w_e * v[src_e] ; out = agg @ Wo.T + bo

Device strategy (8 cores, node-range sharding, three SPMD launches):
  L1 (proj): each core computes q/k/v (fp16) for its node shard.
  L2 (src phase): core c owns edges with src in its shard, laid out in
      128-edge chunks grouped by 128-node src block.  The k rows for each
      edge slot arrive as a host-prepared per-slot int8 stream (contiguous,
      full DMA bandwidth; per-row quantization scales are applied to the
      reduced scores, not the rows).  q rows are expanded per-edge on-chip
      via PE matmuls against streamed one-hot matrices in [feat x slot]
      orientation; the per-head dot products are then a second PE matmul
      against a tiny constant block-diagonal matrix, so no DVE reduction is
      needed.  exp -> per-src-block segment sums via PE matmuls with
      one-hots -> reciprocal -> per-edge softmax weights w_e (output).
  L3 (dest phase): core c owns edges with dest in its shard.  v rows arrive
      as a per-slot int8 stream; weighted rows (w_e * v) are scatter-added
      into per-dest-block agg via PE matmuls with one-hots, then the output
      projection.  No collectives and no device-side gathers anywhere.
  Host between launches does relayout only: assembling tables from L1/L2
  outputs, per-row int8 packing, per-slot stream/one-hot construction, and
  permutation of edge weights between the src- and dest-groupings.
"""

import math
import ml_dtypes
import numpy as np

import concourse.bacc as bacc
import concourse.mybir as mybir
import concourse.tile as tile
from concourse.bass_utils import run_bass_kernel_spmd

# ---------------------------------------------------------------- constants
N = 50000
E = 600000
F = 128
H = 8
Dh = 16
P = 128
C = 8                     # cores
SH = 6272                 # nodes per core, cores 0-6 (49 blocks); core 7: 6096
NB = 49                   # blocks per shard
G = 8                     # chunks per processing group (psum-sized)
KB = 64                   # chunks per stream DMA tile
SG = 12                   # blocks per recip supergroup in L2
F16 = mybir.dt.float16
F8 = mybir.dt.float8e4
F32 = mybir.dt.float32
I8 = mybir.dt.int8


def shard_base(c):
    return c * SH


def shard_len(c):
    return min(N, (c + 1) * SH) - c * SH


# ---------------------------------------------------------------- host prep
class ChunkMap:
    """Uniform chunk structure shared by all cores for one phase.

    Chunks (128 slots each) are block-major: kb[b] chunks for block b; the
    chunk->block map is identical on every core so one program serves all 8."""

    def __init__(self, kb):
        self.kb = [int(x) for x in kb]
        self.chunks = [b for b in range(NB) for _ in range(self.kb[b])]
        self.nch = len(self.chunks)
        self.nslots = self.nch * P
        self.start = np.concatenate([[0], np.cumsum(self.kb)]).astype(int)


def compute_cmap(key, other=None):
    """Global uniform per-block chunk counts for one phase."""
    kb = np.ones(NB, np.int64)
    for c in range(C):
        base, ln = shard_base(c), shard_len(c)
        m = (key >= base) & (key < base + ln)
        cnt = np.bincount((key[m] - base) // P, minlength=NB)
        kb = np.maximum(kb, (cnt + P - 1) // P)
    return ChunkMap(kb)


def group_list(cmap):
    """Processing groups of up to G chunks, block-aligned: (b, g0, gn, cs, ce)."""
    groups = []
    for b in range(NB):
        cs, ce = int(cmap.start[b]), int(cmap.start[b + 1])
        for g0 in range(cs, ce, G):
            groups.append((b, g0, min(G, ce - g0), cs, ce))
    return groups


def chunk_split(cmap, num, den):
    """Mixed-precision chunk routing: groups with gi % den < num take the
    f16 path (Act-assisted 2x multiply), the rest the int8 path (half DMA).
    Returns (f16 flag per chunk, compact offset per chunk, n8, n16)."""
    f16 = np.zeros(cmap.nch, bool)
    for i, (b, g0, gn, cs, ce) in enumerate(group_list(cmap)):
        if i % den < num:
            f16[g0:g0 + gn] = True
    off = np.zeros(cmap.nch, np.int64)
    n8 = n16 = 0
    for ci in range(cmap.nch):
        if f16[ci]:
            off[ci] = n16
            n16 += 1
        else:
            off[ci] = n8
            n8 += 1
    return f16, off, n8, n16


class CorePlan:
    """Per-core slot contents for one phase.  `key` = node defining the block
    (src for L2, dest for L3); `other` = node whose row the slot consumes."""

    def __init__(self, cmap, core, key, other, edge_ids):
        base = shard_base(core)
        nsl = cmap.nslots
        self.slot_local = np.full(nsl, -1, np.int64)
        self.slot_other = np.zeros(nsl, np.int64)
        self.slot_edge = np.full(nsl, -1, np.int64)
        block = (key - base) // P
        for b in range(NB):
            m = block == b
            cnt = int(m.sum())
            if cnt == 0:
                continue
            assert cnt <= cmap.kb[b] * P
            s0 = int(cmap.start[b]) * P
            self.slot_local[s0:s0 + cnt] = key[m] - base - b * P
            self.slot_other[s0:s0 + cnt] = other[m]
            self.slot_edge[s0:s0 + cnt] = edge_ids[m]
        self.cmap = cmap

    def onehot_stream(self, transposed):
        """[128, nch*128] fp8; chunk c at cols c*128:(c+1)*128.
        transposed=False: S [loc, (c,slot)] ; True: ST/TT [slot, (c,loc)].
        Dummy slots are all-zero columns/rows."""
        cm = self.cmap
        out = np.zeros((P, cm.nch * P), dtype=ml_dtypes.float8_e4m3)
        loc = self.slot_local
        sl_all = np.arange(cm.nslots)
        valid = loc >= 0
        ch = sl_all // P
        row = sl_all % P
        if transposed:
            out[row[valid], ch[valid] * P + loc[valid]] = 1.0
        else:
            out[loc[valid], ch[valid] * P + row[valid]] = 1.0
        return out


def row_quant_int8(a16):
    """Per-row symmetric int8 quantization.  Returns (int8 vals, f32 scales)."""
    a = np.asarray(a16, np.float32)
    am = np.abs(a).max(axis=1)
    s = np.where(am > 0, am / 127.0, 1.0).astype(np.float32)
    q = np.clip(np.round(a / s[:, None]), -127, 127).astype(np.int8)
    return q, s


def head_mask_matrix():
    """[128, 8] fp8 block-diagonal ones: B[f, h] = (f // 16 == h)."""
    B = np.zeros((P, H), dtype=ml_dtypes.float8_e4m3)
    for h in range(H):
        B[h * Dh:(h + 1) * Dh, h] = 1.0
    return B


# ---------------------------------------------------------------- L1: projections
def build_l1():
    nc = bacc.Bacc("TRN2", target_bir_lowering=False, num_devices=C)
    xT = nc.dram_tensor("xT", [P, NB * P], F16, kind="ExternalInput")
    wqkv = nc.dram_tensor("wqkv", [P, 3 * P], F16, kind="ExternalInput")
    bqkv = nc.dram_tensor("bqkv", [1, 3 * P], F16, kind="ExternalInput")
    ones = nc.dram_tensor("ones", [1, P], F16, kind="ExternalInput")
    qkv = nc.dram_tensor("qkv", [P, NB * 3 * P], F16, kind="ExternalOutput")

    with tile.TileContext(nc) as tc:
        with tc.tile_pool(name="const", bufs=1) as cpool, \
             tc.tile_pool(name="stage", bufs=3) as spool, \
             tc.tile_pool(name="psum", bufs=4, space="PSUM") as ppool:
            w_sb = cpool.tile([P, 3 * P], F16, tag="w", name="w_sb")
            nc.sync.dma_start(w_sb[:], wqkv[:])
            b_sb = cpool.tile([1, 3 * P], F16, tag="b", name="b_sb")
            nc.sync.dma_start(b_sb[:], bqkv[:])
            ones_sb = cpool.tile([1, P], F16, tag="ones", name="ones_sb")
            nc.sync.dma_start(ones_sb[:], ones[:])
            xt = cpool.tile([P, NB * P], F16, tag="xT", name="xt")
            for i in range(4):
                a = i * 13 * P
                b = min(NB, (i + 1) * 13) * P
                nc.sync.dma_start(xt[:, a:b], xT[:, a:b])
            DB = 7   # blocks per output DMA
            osb = cpool.tile([P, NB * 3 * P], F16, tag="osb", name="osb")
            for b in range(NB):
                ps = ppool.tile([P, 3 * P], F32, tag="proj", name="ps")
                nc.tensor.matmul(ps[:], lhsT=xt[:, b * P:(b + 1) * P],
                                 rhs=w_sb[:], start=True, stop=False)
                nc.tensor.matmul(ps[:], lhsT=ones_sb[:], rhs=b_sb[:],
                                 start=False, stop=True)
                dstsl = osb[:, b * 3 * P:(b + 1) * 3 * P]
                if b % 2 == 0:
                    nc.vector.tensor_copy(dstsl, ps[:])
                else:
                    nc.scalar.copy(dstsl, ps[:])
                if b % DB == DB - 1 or b == NB - 1:
                    a0 = (b // DB) * DB * 3 * P
                    a1 = (b + 1) * 3 * P
                    nc.gpsimd.dma_start(qkv[:, a0:a1], osb[:, a0:a1])
    nc.compile()
    return nc


# ---------------------------------------------------------------- L2: src phase
L2_SPLIT = (1, 4)   # 1/4 of groups take the f16 k path


def build_l2(cmap):
    nch = cmap.nch
    kf16, koff, n8, n16 = chunk_split(cmap, *L2_SPLIT)
    nc = bacc.Bacc("TRN2", target_bir_lowering=False, num_devices=C)
    q_sh = nc.dram_tensor("q_sh", [P, NB * P], F16, kind="ExternalInput")
    kst8 = nc.dram_tensor("kst8", [P, max(n8, 1) * P], I8, kind="ExternalInput")
    kst16 = nc.dram_tensor("kst16", [P, max(n16, 1) * P], F16,
                           kind="ExternalInput")
    srow = nc.dram_tensor("srow", [P, nch], F16, kind="ExternalInput")
    S_st = nc.dram_tensor("S_st", [P, nch * P], F8, kind="ExternalInput")
    ST_st = nc.dram_tensor("ST_st", [P, nch * P], F8, kind="ExternalInput")
    Bm = nc.dram_tensor("Bm", [P, H], F8, kind="ExternalInput")
    exp_out = nc.dram_tensor("exp_out", [P, nch * H], F16, kind="ExternalOutput")
    rec_out = nc.dram_tensor("rec_out", [P, NB * H], F16, kind="ExternalOutput")

    groups = group_list(cmap)
    ng = len(groups)

    with tile.TileContext(nc) as tc:
        with tc.tile_pool(name="res", bufs=1) as rpool, \
             tc.tile_pool(name="kst", bufs=2) as kpool, \
             tc.tile_pool(name="st", bufs=2) as tpool, \
             tc.tile_pool(name="sst", bufs=2) as spool, \
             tc.tile_pool(name="work", bufs=4) as wpool, \
             tc.tile_pool(name="qpsum", bufs=2, space="PSUM") as qpsum, \
             tc.tile_pool(name="spsum", bufs=2, space="PSUM") as spsum, \
             tc.tile_pool(name="gpsum", bufs=2, space="PSUM") as gpsum:
            B_sb = rpool.tile([P, H], F8, tag="B", name="B_sb")
            nc.sync.dma_start(B_sb[:], Bm[:])
            srow_sb = rpool.tile([P, nch], F16, tag="srow", name="srow_sb")
            nc.sync.dma_start(srow_sb[:], srow[:])
            q_sb = rpool.tile([P, NB * P], F16, tag="q", name="q_sb")
            for i in range(4):
                a = (i * NB // 4) * P
                b = ((i + 1) * NB // 4) * P
                nc.sync.dma_start(q_sb[:, a:b], q_sh[:, a:b])
            exp_sb = rpool.tile([P, nch * H], F16, tag="exp", name="exp_sb")
            seg_sb = rpool.tile([P, NB * H], F32, tag="seg", name="seg_sb")
            rec_sb = rpool.tile([P, NB * H], F16, tag="rec", name="rec_sb")

            k_tiles = {}
            k16_tiles = {}
            t_tiles = {}
            s_tiles = {}

            def stream(tiles, pool, dram, ci, width=P, dt=I8, total=None,
                       off=None):
                cc = ci if off is None else int(off[ci])
                tot = nch if total is None else total
                t0 = cc // KB * KB
                if t0 not in tiles:
                    t = pool.tile([P, KB * width], dt, tag=dram.name,
                                  name=f"strm_{dram.name}_{t0}")
                    n = min(KB, tot - t0) * width
                    nc.sync.dma_start(t[:, :n], dram[:, t0 * width:t0 * width + n])
                    tiles[t0] = t
                return tiles[t0], t0, cc

            # Software-pipelined stages, skewed so PE never queue-stalls on a
            # cross-engine dependency:
            #   A(i):   q expansion (PE) + qk multiply (DVE)
            #   B(i-1): score matmuls (PE)
            #   C(i-2): on last group of a block: dequant (DVE), exp (Act),
            #           segment-sum matmuls (PE), seg copy (Act)
            qkTs = {}
            scps = {}

            def stage_a(i):
                b, g0, gn, cs, ce = groups[i]
                qeT = qpsum.tile([P, G * P], F32, tag="qeT", name="qeT")
                ci = g0
                while ci < g0 + gn:
                    st, t0, _ = stream(s_tiles, spool, S_st, ci, dt=F8)
                    # pieces must not cross 512-col psum bank boundaries
                    cj = min(g0 + gn, t0 + KB, g0 + ((ci - g0) // 4 + 1) * 4)
                    nsl = (cj - ci) * P
                    nc.tensor.matmul(
                        qeT[:, (ci - g0) * P:(ci - g0) * P + nsl],
                        lhsT=q_sb[:, b * P:(b + 1) * P],
                        rhs=st[:, (ci - t0) * P:(ci - t0) * P + nsl],
                        start=True, stop=True)
                    ci = cj
                qkT = wpool.tile([P, G * P], F16, tag="qkT", name="qkT")
                f16p = bool(kf16[g0])
                if f16p:
                    qe16 = wpool.tile([P, G * P], F16, tag="qe16", name="qe16")
                    nc.scalar.copy(qe16[:, :gn * P], qeT[:, :gn * P])
                    in0 = qe16
                else:
                    in0 = qeT
                ci = g0
                while ci < g0 + gn:
                    if f16p:
                        kt, t0, cc = stream(k16_tiles, kpool, kst16, ci,
                                            dt=F16, total=n16, off=koff)
                    else:
                        kt, t0, cc = stream(k_tiles, kpool, kst8, ci,
                                            dt=I8, total=n8, off=koff)
                    cj = min(g0 + gn, ci + (t0 + KB - cc))
                    nsl = (cj - ci) * P
                    nc.vector.tensor_mul(
                        qkT[:, (ci - g0) * P:(ci - g0) * P + nsl],
                        in0[:, (ci - g0) * P:(ci - g0) * P + nsl],
                        kt[:, (cc - t0) * P:(cc - t0) * P + nsl])
                    ci = cj
                qkTs[i] = qkT

            def stage_b(i):
                b, g0, gn, cs, ce = groups[i]
                qkT = qkTs.pop(i)
                if g0 == cs:
                    scps[b] = spsum.tile([P, 16 * H], F32, tag="sc",
                                         name="scp")
                scp = scps[b]
                for ci in range(g0, g0 + gn):
                    nc.tensor.matmul(
                        scp[:, (ci - cs) * H:(ci - cs + 1) * H],
                        lhsT=qkT[:, (ci - g0) * P:(ci - g0 + 1) * P],
                        rhs=B_sb[:], start=True, stop=True)

            def stage_c(i):
                b, g0, gn, cs, ce = groups[i]
                if g0 + gn != ce:
                    return
                nb = ce - cs
                scp = scps.pop(b)
                sc16 = wpool.tile([P, 16 * H], F16, tag="sc16", name="sc16")
                nc.vector.tensor_mul(
                    sc16[:, :nb * H].rearrange("p (c h) -> p c h", h=H),
                    scp[:, :nb * H].rearrange("p (c h) -> p c h", h=H),
                    srow_sb[:, cs:ce][:, :, None]
                    .broadcast_to([P, nb, H]))
                nc.scalar.activation(
                    out=exp_sb[:, cs * H:ce * H],
                    in_=sc16[:, :nb * H],
                    func=mybir.ActivationFunctionType.Exp,
                    scale=1.0)
                segp = gpsum.tile([P, H], F32, tag="seg", name="segp")
                for ci in range(cs, ce):
                    tt, t0, _ = stream(t_tiles, tpool, ST_st, ci, dt=F8)
                    nc.tensor.matmul(
                        segp[:],
                        lhsT=tt[:, (ci - t0) * P:(ci - t0 + 1) * P],
                        rhs=exp_sb[:, ci * H:(ci + 1) * H],
                        start=(ci == cs), stop=(ci == ce - 1))
                nc.scalar.copy(seg_sb[:, b * H:(b + 1) * H], segp[:])

            for i in range(ng + 2):
                if i < ng:
                    stage_a(i)
                if 1 <= i <= ng:
                    stage_b(i - 1)
                if 2 <= i <= ng + 1:
                    stage_c(i - 2)
                    bdone = groups[i - 2][0]
                    if groups[i - 2][1] + groups[i - 2][2] == groups[i - 2][4]:
                        # exp_out slice per ~8 finished blocks
                        if bdone % 8 == 7 or bdone == NB - 1:
                            a = int(cmap.start[bdone // 8 * 8]) * H
                            bcol = int(cmap.start[bdone + 1]) * H
                            nc.gpsimd.dma_start(exp_out[:, a:bcol],
                                                exp_sb[:, a:bcol])

            # reciprocal; empty segments (zero-degree locs, padding) get
            # seg+1 so it stays finite — their one-hot columns are all-zero
            # downstream so the value never contributes.
            seg1 = wpool.tile([P, NB * H], F32, tag="seg1", name="seg1")
            nc.vector.scalar_tensor_tensor(
                out=seg1[:], in0=seg_sb[:], scalar=0.0, in1=seg_sb[:],
                op0=mybir.AluOpType.is_le, op1=mybir.AluOpType.add)
            with nc.allow_low_precision(reason="softmax recip fits f16"):
                nc.vector.reciprocal(rec_sb[:], seg1[:])
            nc.gpsimd.dma_start(rec_out[:], rec_sb[:])
    nc.compile()
    return nc


# ---------------------------------------------------------------- L3: dest phase
L3_SPLIT = (2, 5)   # 2/5 of groups take the f16 v path


def build_l3(cmap):
    nch = cmap.nch
    vf16, voff, n8, n16 = chunk_split(cmap, *L3_SPLIT)
    nc = bacc.Bacc("TRN2", target_bir_lowering=False, num_devices=C)
    vst8 = nc.dram_tensor("vst8", [P, max(n8, 1) * P], I8, kind="ExternalInput")
    vst16 = nc.dram_tensor("vst16", [P, max(n16, 1) * P], F16,
                           kind="ExternalInput")
    exp_in = nc.dram_tensor("exp_in", [P, nch * H], F16, kind="ExternalInput")
    srw = nc.dram_tensor("srw", [P, nch * H], F16, kind="ExternalInput")
    TT_st = nc.dram_tensor("TT_st", [P, nch * P], F8, kind="ExternalInput")
    WoT = nc.dram_tensor("WoT", [P, P], F16, kind="ExternalInput")
    bo_r = nc.dram_tensor("bo_r", [1, P], F16, kind="ExternalInput")
    ones = nc.dram_tensor("ones", [1, P], F16, kind="ExternalInput")
    outT = nc.dram_tensor("outT", [P, NB * P], F16, kind="ExternalOutput")

    with tile.TileContext(nc) as tc:
        with tc.tile_pool(name="res", bufs=1) as rpool, \
             tc.tile_pool(name="vstp", bufs=2) as vpool, \
             tc.tile_pool(name="tt", bufs=2) as tpool, \
             tc.tile_pool(name="work", bufs=3) as wpool, \
             tc.tile_pool(name="apsum", bufs=2, space="PSUM") as apsum, \
             tc.tile_pool(name="opsum", bufs=2, space="PSUM") as opsum:
            wo_sb = rpool.tile([P, P], F16, tag="wo", name="wo_sb")
            nc.sync.dma_start(wo_sb[:], WoT[:])
            bo_sb = rpool.tile([1, P], F16, tag="bo", name="bo_sb")
            nc.sync.dma_start(bo_sb[:], bo_r[:])
            ones_sb = rpool.tile([1, P], F16, tag="ones", name="ones_sb")
            nc.sync.dma_start(ones_sb[:], ones[:])
            osb = rpool.tile([P, NB * P], F16, tag="osb", name="osb")
            w_sb = rpool.tile([P, nch * H], F16, tag="w", name="w_sb")
            sv_sb = rpool.tile([P, nch * H], F16, tag="sv", name="sv_sb")
            wsc = rpool.tile([P, nch * H], F16, tag="wsc", name="wsc")

            v_tiles = {}
            v16_tiles = {}
            t_tiles = {}
            w_spans = set()

            def stream(tiles, pool, dram, ci, dt, total=None, off=None):
                cc = ci if off is None else int(off[ci])
                tot = nch if total is None else total
                t0 = cc // KB * KB
                if t0 not in tiles:
                    t = pool.tile([P, KB * P], dt, tag=dram.name,
                                  name=f"strm_{dram.name}_{t0}")
                    n = min(KB, tot - t0) * P
                    nc.sync.dma_start(t[:, :n], dram[:, t0 * P:t0 * P + n])
                    tiles[t0] = t
                return tiles[t0], t0, cc

            def want_wsc(ci):
                # per-edge weight exp * (rec[src] * v-row-scale): lazily DMA'd
                # and computed (packed f16 2x mult) per KB-chunk span
                t0 = ci // KB * KB
                if t0 not in w_spans:
                    w_spans.add(t0)
                    a = t0 * H
                    bcol = min(nch, t0 + KB) * H
                    nc.sync.dma_start(w_sb[:, a:bcol], exp_in[:, a:bcol])
                    nc.sync.dma_start(sv_sb[:, a:bcol], srw[:, a:bcol])
                    nc.vector.tensor_mul(wsc[:, a:bcol], w_sb[:, a:bcol],
                                         sv_sb[:, a:bcol])

            gi = 0
            DB = 7
            for b in range(NB):
                cs, ce = int(cmap.start[b]), int(cmap.start[b + 1])
                aggp = apsum.tile([P, P], F32, tag="agg", name="aggp")
                for g0 in range(cs, ce, G):
                    gn = min(G, ce - g0)
                    wv = wpool.tile([P, G * P], F16, tag="wv", name="wv")
                    want_wsc(g0)
                    want_wsc(g0 + gn - 1)
                    f16p = bool(vf16[g0])
                    erep = None
                    if f16p:
                        # materialize the broadcast weights on Act so the
                        # multiply runs packed-f16 at DVE 2x
                        erep = wpool.tile([P, G * P], F16, tag="erep",
                                          name="erep")
                        nc.scalar.copy(
                            erep[:, :gn * P]
                            .rearrange("p (c h d) -> p c h d", h=H, d=Dh),
                            wsc[:, g0 * H:(g0 + gn) * H]
                            .rearrange("p (c h) -> p c h", h=H)[:, :, :, None]
                            .broadcast_to([P, gn, H, Dh]))
                    ci = g0
                    while ci < g0 + gn:
                        if f16p:
                            vt, t0, cc = stream(v16_tiles, vpool, vst16, ci,
                                                F16, total=n16, off=voff)
                        else:
                            vt, t0, cc = stream(v_tiles, vpool, vst8, ci,
                                                I8, total=n8, off=voff)
                        cj = min(g0 + gn, ci + (t0 + KB - cc))
                        nn = cj - ci
                        if f16p:
                            nc.vector.tensor_mul(
                                wv[:, (ci - g0) * P:(ci - g0 + nn) * P],
                                vt[:, (cc - t0) * P:(cc - t0 + nn) * P],
                                erep[:, (ci - g0) * P:(ci - g0 + nn) * P])
                        else:
                            nc.vector.tensor_mul(
                                wv[:, (ci - g0) * P:(ci - g0 + nn) * P]
                                .rearrange("p (c h d) -> p c h d", h=H, d=Dh),
                                vt[:, (cc - t0) * P:(cc - t0 + nn) * P]
                                .rearrange("p (c h d) -> p c h d", h=H, d=Dh),
                                wsc[:, ci * H:(ci + nn) * H]
                                .rearrange("p (c h) -> p c h", h=H)
                                [:, :, :, None]
                                .broadcast_to([P, nn, H, Dh]))
                        ci = cj
                    gi += 1
                    for ci in range(g0, g0 + gn):
                        tt, t0, _ = stream(t_tiles, tpool, TT_st, ci, F8)
                        nc.tensor.matmul(
                            aggp[:],
                            lhsT=wv[:, (ci - g0) * P:(ci - g0 + 1) * P],
                            rhs=tt[:, (ci - t0) * P:(ci - t0 + 1) * P],
                            start=(ci == cs), stop=(ci == ce - 1))
                agg16 = wpool.tile([P, P], F16, tag="agg16", name="agg16")
                nc.scalar.copy(agg16[:], aggp[:])
                outp = opsum.tile([P, P], F32, tag="outp", name="outp")
                nc.tensor.matmul(outp[:], lhsT=wo_sb[:], rhs=agg16[:],
                                 start=True, stop=False)
                nc.tensor.matmul(outp[:], lhsT=bo_sb[:], rhs=ones_sb[:],
                                 start=False, stop=True)
                nc.scalar.copy(osb[:, b * P:(b + 1) * P], outp[:])
                if b % DB == DB - 1 or b == NB - 1:
                    a0 = (b // DB) * DB * P
                    a1 = (b + 1) * P
                    nc.gpsimd.dma_start(outT[:, a0:a1], osb[:, a0:a1])
    nc.compile()
    return nc


# ---------------------------------------------------------------- orchestration
def _prep_weights(Wq, bq, Wk, bk, Wv, bv, Wo, bo):
    w16 = {k: np.asarray(v, np.float32).astype(np.float16)
           for k, v in (("Wq", Wq), ("Wk", Wk), ("Wv", Wv), ("Wo", Wo))}
    b16 = {k: np.asarray(v, np.float32).astype(np.float16)
           for k, v in (("bq", bq), ("bk", bk), ("bv", bv), ("bo", bo))}
    return w16, b16


def kernel(node_features, edge_index, Wq, bq, Wk, bk, Wv, bv, Wo, bo):
    node_features = np.asarray(node_features, np.float32)
    edge_index = np.asarray(edge_index)
    src, dst = edge_index[0].astype(np.int64), edge_index[1].astype(np.int64)
    x16 = node_features.astype(np.float16)
    w16, b16 = _prep_weights(Wq, bq, Wk, bk, Wv, bv, Wo, bo)
    ones_row = np.ones((1, P), np.float16)
    cores = list(range(C))
    eids = np.arange(E, dtype=np.int64)

    # ---------------- L1
    nc1 = build_l1()
    in1 = []
    for c in cores:
        base, ln = shard_base(c), shard_len(c)
        xt = np.zeros((P, NB * P), np.float16)
        xt[:, :ln] = x16[base:base + ln].T
        in1.append(dict(
            xT=xt,
            wqkv=np.concatenate([w16["Wq"].T, w16["Wk"].T, w16["Wv"].T],
                                axis=1).copy(),
            bqkv=np.concatenate([b16["bq"], b16["bk"], b16["bv"]])
            .reshape(1, 3 * P), ones=ones_row))
    r1 = run_bass_kernel_spmd(nc1, in1, core_ids=cores)

    q_shs = []
    k_all = np.zeros((N, P), np.float16)
    v_all = np.zeros((N, P), np.float16)
    for c in cores:
        base, ln = shard_base(c), shard_len(c)
        blob = r1.results[c]["qkv"].reshape(P, NB, 3, P)
        q_shs.append(np.ascontiguousarray(blob[:, :, 0, :].reshape(P, NB * P)))
        k_sh = blob[:, :, 1, :].transpose(1, 0, 2).reshape(NB * P, P)
        v_sh = blob[:, :, 2, :].transpose(1, 0, 2).reshape(NB * P, P)
        k_all[base:base + ln] = k_sh[:ln]
        v_all[base:base + ln] = v_sh[:ln]

    k8, krs = row_quant_int8(k_all)
    v8, vrs = row_quant_int8(v_all)

    # ---------------- L2
    cmap2 = compute_cmap(src)
    plans2 = []
    for c in cores:
        base, ln = shard_base(c), shard_len(c)
        m = (src >= base) & (src < base + ln)
        plans2.append(CorePlan(cmap2, c, src[m], dst[m], eids[m]))

    nc2 = build_l2(cmap2)
    Bmat = head_mask_matrix()
    kf16, koff, kn8, kn16 = chunk_split(cmap2, *L2_SPLIT)
    in2 = []
    for c in cores:
        pl = plans2[c]
        nch = cmap2.nch
        oth = pl.slot_other.reshape(nch, P)
        k8_rows = k8[oth[~kf16]]                       # [n8, P, P] int8
        k16_rows = k_all[oth[kf16]]                    # [n16, P, P] f16
        kst8 = k8_rows.transpose(2, 0, 1).reshape(P, -1).copy() \
            if kn8 else np.zeros((P, P), np.int8)
        kst16 = k16_rows.transpose(2, 0, 1).reshape(P, -1).copy() \
            if kn16 else np.zeros((P, P), np.float16)
        valid = (pl.slot_edge >= 0).astype(np.float32)
        scale = np.where(np.repeat(kf16, P), 1.0, krs[pl.slot_other])
        srow_v = (scale * valid * 0.25).astype(np.float16)
        in2.append(dict(
            q_sh=q_shs[c], kst8=kst8, kst16=kst16,
            srow=np.ascontiguousarray(srow_v.reshape(nch, P).T),
            S_st=pl.onehot_stream(False), ST_st=pl.onehot_stream(True),
            Bm=Bmat))
    r2 = run_bass_kernel_spmd(nc2, in2, core_ids=cores)

    exp_edge = np.zeros((E, H), np.float16)
    rec_all = np.zeros((N, H), np.float16)
    for c in cores:
        pl = plans2[c]
        e_flat = r2.results[c]["exp_out"].reshape(P, cmap2.nch, H) \
            .transpose(1, 0, 2).reshape(cmap2.nslots, H)
        real = pl.slot_edge >= 0
        exp_edge[pl.slot_edge[real]] = e_flat[real]
        base, ln = shard_base(c), shard_len(c)
        rec_sh = r2.results[c]["rec_out"].reshape(P, NB, H) \
            .transpose(1, 0, 2).reshape(NB * P, H)
        rec_all[base:base + ln] = rec_sh[:ln]

    # ---------------- L3
    cmap3 = compute_cmap(dst)
    plans3 = []
    for c in cores:
        base, ln = shard_base(c), shard_len(c)
        m = (dst >= base) & (dst < base + ln)
        plans3.append(CorePlan(cmap3, c, dst[m], src[m], eids[m]))

    nc3 = build_l3(cmap3)
    vf16, voff, vn8, vn16 = chunk_split(cmap3, *L3_SPLIT)
    in3 = []
    for c in cores:
        pl = plans3[c]
        nch = cmap3.nch
        oth = pl.slot_other.reshape(nch, P)
        v8_rows = v8[oth[~vf16]]
        v16_rows = v_all[oth[vf16]]
        vst8 = v8_rows.transpose(1, 0, 2).reshape(P, -1).copy() \
            if vn8 else np.zeros((P, P), np.int8)
        vst16 = v16_rows.transpose(1, 0, 2).reshape(P, -1).copy() \
            if vn16 else np.zeros((P, P), np.float16)
        e_slots = np.zeros((cmap3.nslots, H), np.float16)
        real = pl.slot_edge >= 0
        e_slots[real] = exp_edge[pl.slot_edge[real]]
        # combined per-slot scale: softmax denominator recip at the src node
        # times the src v-row int8 scale (1 for f16 chunks, 0 on padding)
        vscale = np.where(np.repeat(vf16, P), 1.0, vrs[pl.slot_other])
        srw_v = (rec_all[pl.slot_other].astype(np.float32) *
                 (vscale * real.astype(np.float32))[:, None]) \
            .astype(np.float16)
        in3.append(dict(
            vst8=vst8, vst16=vst16,
            exp_in=np.ascontiguousarray(
                e_slots.reshape(nch, P, H).transpose(1, 0, 2)
                .reshape(P, nch * H)),
            srw=np.ascontiguousarray(
                srw_v.reshape(nch, P, H).transpose(1, 0, 2)
                .reshape(P, nch * H)),
            TT_st=pl.onehot_stream(True),
            WoT=w16["Wo"].T.copy(),
            bo_r=b16["bo"].reshape(1, P), ones=ones_row))
    r3 = run_bass_kernel_spmd(nc3, in3, core_ids=cores)

    out = np.zeros((N, F), np.float32)
    for c in cores:
        base, ln = shard_base(c), shard_len(c)
        o = r3.results[c]["outT"].reshape(P, NB, P).transpose(1, 2, 0) \
            .reshape(NB * P, P)
        out[base:base + ln] = o[:ln].astype(np.float32)
    return out


# revision 49
# speedup vs baseline: 1.0817x; 1.0743x over previous
"""Trainium2 Bass kernel for nn_EnhancedReflectiveCognitiveGraph (GNN edge-softmax attention).

Math (see reference):
  q/k/v = x @ W{q,k,v}.T + b ; per-edge scores s_e = <q[src_e], k[dest_e]>_head / 4
  softmax over edges sharing src (max-subtraction skipped: scores ~ N(0,1) so
  exp never overflows and the weights are mathematically identical)
  agg[dest] += w_e * v[src_e] ; out = agg @ Wo.T + bo

Device strategy (8 cores, node-range sharding, three SPMD launches):
  L1 (proj): each core computes q/k/v (fp16) for its node shard.
  L2 (src phase): core c owns edges with src in its shard, laid out in
      128-edge chunks grouped by 128-node src block.  The k rows for each
      edge slot arrive as a host-prepared per-slot int8 stream (contiguous,
      full DMA bandwidth; per-row quantization scales are applied to the
      reduced scores, not the rows).  q rows are expanded per-edge on-chip
      via PE matmuls against streamed one-hot matrices in [feat x slot]
      orientation; the per-head dot products are then a second PE matmul
      against a tiny constant block-diagonal matrix, so no DVE reduction is
      needed.  exp -> per-src-block segment sums via PE matmuls with
      one-hots -> reciprocal -> per-edge softmax weights w_e (output).
  L3 (dest phase): core c owns edges with dest in its shard.  v rows arrive
      as a per-slot int8 stream; weighted rows (w_e * v) are scatter-added
      into per-dest-block agg via PE matmuls with one-hots, then the output
      projection.  No collectives and no device-side gathers anywhere.
  Host between launches does relayout only: assembling tables from L1/L2
  outputs, per-row int8 packing, per-slot stream/one-hot construction, and
  permutation of edge weights between the src- and dest-groupings.
"""

import math
import ml_dtypes
import numpy as np

import concourse.bacc as bacc
import concourse.mybir as mybir
import concourse.tile as tile
from concourse.bass_utils import run_bass_kernel_spmd

# ---------------------------------------------------------------- constants
N = 50000
E = 600000
F = 128
H = 8
Dh = 16
P = 128
C = 8                     # cores
SH = 6272                 # nodes per core, cores 0-6 (49 blocks); core 7: 6096
NB = 49                   # blocks per shard
G = 8                     # chunks per processing group (psum-sized)
KB = 32                   # chunks per stream DMA tile (L2)
KB3 = 64                  # chunks per stream DMA tile (L3)
SG = 12                   # blocks per recip supergroup in L2
F16 = mybir.dt.float16
F8 = mybir.dt.float8e4
F32 = mybir.dt.float32
I8 = mybir.dt.int8


def shard_base(c):
    return c * SH


def shard_len(c):
    return min(N, (c + 1) * SH) - c * SH


# ---------------------------------------------------------------- host prep
class ChunkMap:
    """Uniform chunk structure shared by all cores for one phase.

    Chunks (128 slots each) are block-major: kb[b] chunks for block b; the
    chunk->block map is identical on every core so one program serves all 8."""

    def __init__(self, kb):
        self.kb = [int(x) for x in kb]
        self.chunks = [b for b in range(NB) for _ in range(self.kb[b])]
        self.nch = len(self.chunks)
        self.nslots = self.nch * P
        self.start = np.concatenate([[0], np.cumsum(self.kb)]).astype(int)


def compute_cmap(key, other=None):
    """Global uniform per-block chunk counts for one phase."""
    kb = np.ones(NB, np.int64)
    for c in range(C):
        base, ln = shard_base(c), shard_len(c)
        m = (key >= base) & (key < base + ln)
        cnt = np.bincount((key[m] - base) // P, minlength=NB)
        kb = np.maximum(kb, (cnt + P - 1) // P)
    return ChunkMap(kb)


def group_list(cmap):
    """Processing groups of up to G chunks, block-aligned: (b, g0, gn, cs, ce)."""
    groups = []
    for b in range(NB):
        cs, ce = int(cmap.start[b]), int(cmap.start[b + 1])
        for g0 in range(cs, ce, G):
            groups.append((b, g0, min(G, ce - g0), cs, ce))
    return groups


def chunk_split(cmap, num, den, tail=False, shift=0):
    """Mixed-precision chunk routing: `num` of every `den` groups take the
    f16 path (Act-assisted 2x multiply), the rest the int8 path (half DMA).
    `tail` places the f16 groups at the end of each cycle (keeps the large
    f16 stream tiles off the pipeline ramp).
    Returns (f16 flag per chunk, compact offset per chunk, n8, n16)."""
    f16 = np.zeros(cmap.nch, bool)
    for i, (b, g0, gn, cs, ce) in enumerate(group_list(cmap)):
        sel = (i % den >= den - num) if tail else ((i - shift) % den < num)
        if sel:
            f16[g0:g0 + gn] = True
    off = np.zeros(cmap.nch, np.int64)
    n8 = n16 = 0
    for ci in range(cmap.nch):
        if f16[ci]:
            off[ci] = n16
            n16 += 1
        else:
            off[ci] = n8
            n8 += 1
    return f16, off, n8, n16


class CorePlan:
    """Per-core slot contents for one phase.  `key` = node defining the block
    (src for L2, dest for L3); `other` = node whose row the slot consumes."""

    def __init__(self, cmap, core, key, other, edge_ids):
        base = shard_base(core)
        nsl = cmap.nslots
        self.slot_local = np.full(nsl, -1, np.int64)
        self.slot_other = np.zeros(nsl, np.int64)
        self.slot_edge = np.full(nsl, -1, np.int64)
        block = (key - base) // P
        for b in range(NB):
            m = block == b
            cnt = int(m.sum())
            if cnt == 0:
                continue
            assert cnt <= cmap.kb[b] * P
            s0 = int(cmap.start[b]) * P
            self.slot_local[s0:s0 + cnt] = key[m] - base - b * P
            self.slot_other[s0:s0 + cnt] = other[m]
            self.slot_edge[s0:s0 + cnt] = edge_ids[m]
        self.cmap = cmap

    def onehot_stream(self, transposed):
        """[128, nch*128] fp8; chunk c at cols c*128:(c+1)*128.
        transposed=False: S [loc, (c,slot)] ; True: ST/TT [slot, (c,loc)].
        Dummy slots are all-zero columns/rows."""
        cm = self.cmap
        out = np.zeros((P, cm.nch * P), dtype=ml_dtypes.float8_e4m3)
        loc = self.slot_local
        sl_all = np.arange(cm.nslots)
        valid = loc >= 0
        ch = sl_all // P
        row = sl_all % P
        if transposed:
            out[row[valid], ch[valid] * P + loc[valid]] = 1.0
        else:
            out[loc[valid], ch[valid] * P + row[valid]] = 1.0
        return out


def row_quant_int8(a16):
    """Per-row symmetric int8 quantization.  Returns (int8 vals, f32 scales)."""
    a = np.asarray(a16, np.float32)
    am = np.abs(a).max(axis=1)
    s = np.where(am > 0, am / 127.0, 1.0).astype(np.float32)
    q = np.clip(np.round(a / s[:, None]), -127, 127).astype(np.int8)
    return q, s


def head_mask_matrix():
    """[128, 8] fp8 block-diagonal ones: B[f, h] = (f // 16 == h)."""
    B = np.zeros((P, H), dtype=ml_dtypes.float8_e4m3)
    for h in range(H):
        B[h * Dh:(h + 1) * Dh, h] = 1.0
    return B


# ---------------------------------------------------------------- L1: projections
def build_l1():
    nc = bacc.Bacc("TRN2", target_bir_lowering=False, num_devices=C)
    xT = nc.dram_tensor("xT", [P, NB * P], F16, kind="ExternalInput")
    wqkv = nc.dram_tensor("wqkv", [P, 3 * P], F16, kind="ExternalInput")
    bqkv = nc.dram_tensor("bqkv", [1, 3 * P], F16, kind="ExternalInput")
    ones = nc.dram_tensor("ones", [1, P], F16, kind="ExternalInput")
    qkv = nc.dram_tensor("qkv", [P, NB * 3 * P], F16, kind="ExternalOutput")

    with tile.TileContext(nc) as tc:
        with tc.tile_pool(name="const", bufs=1) as cpool, \
             tc.tile_pool(name="stage", bufs=3) as spool, \
             tc.tile_pool(name="psum", bufs=4, space="PSUM") as ppool:
            w_sb = cpool.tile([P, 3 * P], F16, tag="w", name="w_sb")
            nc.sync.dma_start(w_sb[:], wqkv[:])
            b_sb = cpool.tile([1, 3 * P], F16, tag="b", name="b_sb")
            nc.sync.dma_start(b_sb[:], bqkv[:])
            ones_sb = cpool.tile([1, P], F16, tag="ones", name="ones_sb")
            nc.sync.dma_start(ones_sb[:], ones[:])
            xt = cpool.tile([P, NB * P], F16, tag="xT", name="xt")
            for i in range(4):
                a = i * 13 * P
                b = min(NB, (i + 1) * 13) * P
                nc.sync.dma_start(xt[:, a:b], xT[:, a:b])
            DB = 7   # blocks per output DMA
            osb = cpool.tile([P, NB * 3 * P], F16, tag="osb", name="osb")
            for b in range(NB):
                ps = ppool.tile([P, 3 * P], F32, tag="proj", name="ps")
                nc.tensor.matmul(ps[:], lhsT=xt[:, b * P:(b + 1) * P],
                                 rhs=w_sb[:], start=True, stop=False)
                nc.tensor.matmul(ps[:], lhsT=ones_sb[:], rhs=b_sb[:],
                                 start=False, stop=True)
                dstsl = osb[:, b * 3 * P:(b + 1) * 3 * P]
                if b % 2 == 0:
                    nc.vector.tensor_copy(dstsl, ps[:])
                else:
                    nc.scalar.copy(dstsl, ps[:])
                if b % DB == DB - 1 or b == NB - 1:
                    a0 = (b // DB) * DB * 3 * P
                    a1 = (b + 1) * 3 * P
                    nc.gpsimd.dma_start(qkv[:, a0:a1], osb[:, a0:a1])
    nc.compile()
    return nc


# ---------------------------------------------------------------- L2: src phase
L2_SPLIT = (1, 4)   # 1/4 of groups take the f16 k path


def build_l2(cmap):
    nch = cmap.nch
    kf16, koff, n8, n16 = chunk_split(cmap, *L2_SPLIT, tail=True)
    nc = bacc.Bacc("TRN2", target_bir_lowering=False, num_devices=C)
    q_sh = nc.dram_tensor("q_sh", [P, NB * P], F16, kind="ExternalInput")
    kst8 = nc.dram_tensor("kst8", [P, max(n8, 1) * P], I8, kind="ExternalInput")
    kst16 = nc.dram_tensor("kst16", [P, max(n16, 1) * P], F16,
                           kind="ExternalInput")
    srow = nc.dram_tensor("srow", [P, nch], F16, kind="ExternalInput")
    S_st = nc.dram_tensor("S_st", [P, nch * P], F8, kind="ExternalInput")
    ST_st = nc.dram_tensor("ST_st", [P, nch * P], F8, kind="ExternalInput")
    Bm = nc.dram_tensor("Bm", [P, H], F8, kind="ExternalInput")
    exp_out = nc.dram_tensor("exp_out", [P, nch * H], F16, kind="ExternalOutput")
    rec_out = nc.dram_tensor("rec_out", [P, NB * H], F16, kind="ExternalOutput")

    groups = group_list(cmap)
    ng = len(groups)

    with tile.TileContext(nc) as tc:
        with tc.tile_pool(name="res", bufs=1) as rpool, \
             tc.tile_pool(name="kst", bufs=3) as kpool, \
             tc.tile_pool(name="st", bufs=3) as tpool, \
             tc.tile_pool(name="sst", bufs=3) as spool, \
             tc.tile_pool(name="work", bufs=6) as wpool, \
             tc.tile_pool(name="qpsum", bufs=3, space="PSUM") as qpsum, \
             tc.tile_pool(name="bpsum", bufs=2, space="PSUM") as bpsum:
            B_sb = rpool.tile([P, H], F8, tag="B", name="B_sb")
            nc.sync.dma_start(B_sb[:], Bm[:])
            srow_sb = rpool.tile([P, nch], F16, tag="srow", name="srow_sb")
            nc.sync.dma_start(srow_sb[:], srow[:])
            q_sb = rpool.tile([P, NB * P], F16, tag="q", name="q_sb")
            q_spans = set()

            def want_q(b):
                i = b * 4 // NB
                if i not in q_spans:
                    q_spans.add(i)
                    a = (i * NB + 3) // 4 * P
                    bb = ((i + 1) * NB + 3) // 4 * P
                    nc.sync.dma_start(q_sb[:, a:bb], q_sh[:, a:bb])
            exp_sb = rpool.tile([P, nch * H], F16, tag="exp", name="exp_sb")
            seg_sb = rpool.tile([P, NB * H], F32, tag="seg", name="seg_sb")
            rec_sb = rpool.tile([P, NB * H], F16, tag="rec", name="rec_sb")

            k_tiles = {}
            k16_tiles = {}
            t_tiles = {}
            s_tiles = {}

            def stream(tiles, pool, dram, ci, width=P, dt=I8, total=None,
                       off=None):
                cc = ci if off is None else int(off[ci])
                tot = nch if total is None else total
                t0 = cc // KB * KB
                if t0 not in tiles:
                    t = pool.tile([P, KB * width], dt, tag=dram.name,
                                  name=f"strm_{dram.name}_{t0}")
                    n = min(KB, tot - t0) * width
                    nc.sync.dma_start(t[:, :n], dram[:, t0 * width:t0 * width + n])
                    tiles[t0] = t
                return tiles[t0], t0, cc

            # Software-pipelined stages, skewed so PE never queue-stalls on a
            # cross-engine dependency:
            #   A(i):   q expansion (PE) + qk multiply (DVE)
            #   B(i-1): score matmuls (PE)
            #   C(i-2): on last group of a block: dequant (DVE), exp (Act),
            #           segment-sum matmuls (PE), seg copy (Act)
            qkTs = {}
            scps = {}

            def prefetch(i):
                # touch the stream spans a group ahead so span-boundary DMA
                # latency never stalls the compute pipeline
                b, g0, gn, cs, ce = groups[i]
                ce2 = min(g0 + gn + G, nch) - 1
                stream(s_tiles, spool, S_st, ce2, dt=F8)
                if kf16[ce2]:
                    stream(k16_tiles, kpool, kst16, ce2, dt=F16, total=n16,
                           off=koff)
                else:
                    stream(k_tiles, kpool, kst8, ce2, dt=I8, total=n8,
                           off=koff)
                stream(t_tiles, tpool, ST_st, ce2, dt=F8)

            def stage_a(i):
                b, g0, gn, cs, ce = groups[i]
                want_q(b)
                want_q(min(b + 3, NB - 1))
                qeT = qpsum.tile([P, G * P], F32, tag="qeT", name="qeT")
                ci = g0
                while ci < g0 + gn:
                    st, t0, _ = stream(s_tiles, spool, S_st, ci, dt=F8)
                    # pieces must not cross 512-col psum bank boundaries
                    cj = min(g0 + gn, t0 + KB, g0 + ((ci - g0) // 4 + 1) * 4)
                    nsl = (cj - ci) * P
                    nc.tensor.matmul(
                        qeT[:, (ci - g0) * P:(ci - g0) * P + nsl],
                        lhsT=q_sb[:, b * P:(b + 1) * P],
                        rhs=st[:, (ci - t0) * P:(ci - t0) * P + nsl],
                        start=True, stop=True)
                    ci = cj
                qkT = wpool.tile([P, G * P], F16, tag="qkT", name="qkT")
                f16p = bool(kf16[g0])
                if f16p:
                    qe16 = wpool.tile([P, G * P], F16, tag="qe16", name="qe16")
                    nc.scalar.copy(qe16[:, :gn * P], qeT[:, :gn * P])
                    in0 = qe16
                else:
                    in0 = qeT
                ci = g0
                while ci < g0 + gn:
                    if f16p:
                        kt, t0, cc = stream(k16_tiles, kpool, kst16, ci,
                                            dt=F16, total=n16, off=koff)
                    else:
                        kt, t0, cc = stream(k_tiles, kpool, kst8, ci,
                                            dt=I8, total=n8, off=koff)
                    cj = min(g0 + gn, ci + (t0 + KB - cc))
                    nsl = (cj - ci) * P
                    nc.vector.tensor_mul(
                        qkT[:, (ci - g0) * P:(ci - g0) * P + nsl],
                        in0[:, (ci - g0) * P:(ci - g0) * P + nsl],
                        kt[:, (cc - t0) * P:(cc - t0) * P + nsl])
                    ci = cj
                qkTs[i] = qkT

            def stage_b(i):
                b, g0, gn, cs, ce = groups[i]
                qkT = qkTs.pop(i)
                if g0 == cs:
                    scps[b] = bpsum.tile([P, 17 * H], F32, tag="blk",
                                         name="blkps")
                scp = scps[b]
                for ci in range(g0, g0 + gn):
                    nc.tensor.matmul(
                        scp[:, (ci - cs) * H:(ci - cs + 1) * H],
                        lhsT=qkT[:, (ci - g0) * P:(ci - g0 + 1) * P],
                        rhs=B_sb[:], start=True, stop=True)

            def stage_c(i):
                b, g0, gn, cs, ce = groups[i]
                if g0 + gn != ce:
                    return
                nb = ce - cs
                blkps = scps.pop(b)
                scp = blkps
                sc16 = wpool.tile([P, 16 * H], F16, tag="sc16", name="sc16")
                nc.vector.tensor_mul(
                    sc16[:, :nb * H].rearrange("p (c h) -> p c h", h=H),
                    scp[:, :nb * H].rearrange("p (c h) -> p c h", h=H),
                    srow_sb[:, cs:ce][:, :, None]
                    .broadcast_to([P, nb, H]))
                nc.scalar.activation(
                    out=exp_sb[:, cs * H:ce * H],
                    in_=sc16[:, :nb * H],
                    func=mybir.ActivationFunctionType.Exp,
                    scale=1.0)
                segp = blkps[:, 16 * H:17 * H]
                for ci in range(cs, ce):
                    tt, t0, _ = stream(t_tiles, tpool, ST_st, ci, dt=F8)
                    nc.tensor.matmul(
                        segp,
                        lhsT=tt[:, (ci - t0) * P:(ci - t0 + 1) * P],
                        rhs=exp_sb[:, ci * H:(ci + 1) * H],
                        start=(ci == cs), stop=(ci == ce - 1))
                nc.scalar.copy(seg_sb[:, b * H:(b + 1) * H], segp)

            for i in range(ng + 2):
                if i < ng:
                    stage_a(i)
                    prefetch(i)
                if 1 <= i <= ng:
                    stage_b(i - 1)
                if 2 <= i <= ng + 1:
                    stage_c(i - 2)
                    bdone = groups[i - 2][0]
                    if groups[i - 2][1] + groups[i - 2][2] == groups[i - 2][4]:
                        # exp_out slice per ~8 finished blocks
                        if bdone % 8 == 7 or bdone == NB - 1:
                            a = int(cmap.start[bdone // 8 * 8]) * H
                            bcol = int(cmap.start[bdone + 1]) * H
                            nc.gpsimd.dma_start(exp_out[:, a:bcol],
                                                exp_sb[:, a:bcol])

            # reciprocal; empty segments (zero-degree locs, padding) get
            # seg+1 so it stays finite — their one-hot columns are all-zero
            # downstream so the value never contributes.
            seg1 = wpool.tile([P, NB * H], F32, tag="seg1", name="seg1")
            nc.vector.scalar_tensor_tensor(
                out=seg1[:], in0=seg_sb[:], scalar=0.0, in1=seg_sb[:],
                op0=mybir.AluOpType.is_le, op1=mybir.AluOpType.add)
            with nc.allow_low_precision(reason="softmax recip fits f16"):
                nc.vector.reciprocal(rec_sb[:], seg1[:])
            nc.gpsimd.dma_start(rec_out[:], rec_sb[:])
    nc.compile()
    return nc


# ---------------------------------------------------------------- L3: dest phase
L3_SPLIT = (2, 5)   # 2/5 of groups take the f16 v path


def build_l3(cmap):
    nch = cmap.nch
    vf16, voff, n8, n16 = chunk_split(cmap, *L3_SPLIT)
    nc = bacc.Bacc("TRN2", target_bir_lowering=False, num_devices=C)
    vst8 = nc.dram_tensor("vst8", [P, max(n8, 1) * P], I8, kind="ExternalInput")
    vst16 = nc.dram_tensor("vst16", [P, max(n16, 1) * P], F16,
                           kind="ExternalInput")
    exp_in = nc.dram_tensor("exp_in", [P, nch * H], F16, kind="ExternalInput")
    srw = nc.dram_tensor("srw", [P, nch * H], F16, kind="ExternalInput")
    TT_st = nc.dram_tensor("TT_st", [P, nch * P], F8, kind="ExternalInput")
    WoT = nc.dram_tensor("WoT", [P, P], F16, kind="ExternalInput")
    bo_r = nc.dram_tensor("bo_r", [1, P], F16, kind="ExternalInput")
    ones = nc.dram_tensor("ones", [1, P], F16, kind="ExternalInput")
    outT = nc.dram_tensor("outT", [P, NB * P], F16, kind="ExternalOutput")

    with tile.TileContext(nc) as tc:
        with tc.tile_pool(name="res", bufs=1) as rpool, \
             tc.tile_pool(name="vstp", bufs=2) as vpool, \
             tc.tile_pool(name="tt", bufs=2) as tpool, \
             tc.tile_pool(name="work", bufs=3) as wpool, \
             tc.tile_pool(name="apsum", bufs=2, space="PSUM") as apsum, \
             tc.tile_pool(name="opsum", bufs=2, space="PSUM") as opsum:
            wo_sb = rpool.tile([P, P], F16, tag="wo", name="wo_sb")
            nc.sync.dma_start(wo_sb[:], WoT[:])
            bo_sb = rpool.tile([1, P], F16, tag="bo", name="bo_sb")
            nc.sync.dma_start(bo_sb[:], bo_r[:])
            ones_sb = rpool.tile([1, P], F16, tag="ones", name="ones_sb")
            nc.sync.dma_start(ones_sb[:], ones[:])
            osb = rpool.tile([P, NB * P], F16, tag="osb", name="osb")
            w_sb = rpool.tile([P, nch * H], F16, tag="w", name="w_sb")
            sv_sb = rpool.tile([P, nch * H], F16, tag="sv", name="sv_sb")
            wsc = rpool.tile([P, nch * H], F16, tag="wsc", name="wsc")

            v_tiles = {}
            v16_tiles = {}
            t_tiles = {}
            w_spans = set()

            def stream(tiles, pool, dram, ci, dt, total=None, off=None):
                cc = ci if off is None else int(off[ci])
                tot = nch if total is None else total
                t0 = cc // KB3 * KB3
                if t0 not in tiles:
                    t = pool.tile([P, KB3 * P], dt, tag=dram.name,
                                  name=f"strm_{dram.name}_{t0}")
                    n = min(KB3, tot - t0) * P
                    nc.sync.dma_start(t[:, :n], dram[:, t0 * P:t0 * P + n])
                    tiles[t0] = t
                return tiles[t0], t0, cc

            def want_wsc(ci):
                # per-edge weight exp * (rec[src] * v-row-scale): lazily DMA'd
                # and computed (packed f16 2x mult) per KB3-chunk span
                t0 = ci // KB3 * KB3
                if t0 not in w_spans:
                    w_spans.add(t0)
                    a = t0 * H
                    bcol = min(nch, t0 + KB3) * H
                    nc.sync.dma_start(w_sb[:, a:bcol], exp_in[:, a:bcol])
                    nc.sync.dma_start(sv_sb[:, a:bcol], srw[:, a:bcol])
                    nc.vector.tensor_mul(wsc[:, a:bcol], w_sb[:, a:bcol],
                                         sv_sb[:, a:bcol])

            DB = 7
            groups = group_list(cmap)
            ng = len(groups)
            wvs = {}
            aggps = {}

            def stage_v(i):
                b, g0, gn, cs, ce = groups[i]
                wv = wpool.tile([P, G * P], F16, tag="wv", name="wv")
                want_wsc(g0)
                want_wsc(g0 + gn - 1)
                f16p = bool(vf16[g0])
                erep = None
                if f16p:
                    # materialize the broadcast weights on Act so the
                    # multiply runs packed-f16 at DVE 2x
                    erep = wpool.tile([P, G * P], F16, tag="erep",
                                      name="erep")
                    nc.scalar.copy(
                        erep[:, :gn * P]
                        .rearrange("p (c h d) -> p c h d", h=H, d=Dh),
                        wsc[:, g0 * H:(g0 + gn) * H]
                        .rearrange("p (c h) -> p c h", h=H)[:, :, :, None]
                        .broadcast_to([P, gn, H, Dh]))
                ci = g0
                while ci < g0 + gn:
                    if f16p:
                        vt, t0, cc = stream(v16_tiles, vpool, vst16, ci,
                                            F16, total=n16, off=voff)
                    else:
                        vt, t0, cc = stream(v_tiles, vpool, vst8, ci,
                                            I8, total=n8, off=voff)
                    cj = min(g0 + gn, ci + (t0 + KB3 - cc))
                    nn = cj - ci
                    if f16p:
                        nc.vector.tensor_mul(
                            wv[:, (ci - g0) * P:(ci - g0 + nn) * P],
                            vt[:, (cc - t0) * P:(cc - t0 + nn) * P],
                            erep[:, (ci - g0) * P:(ci - g0 + nn) * P])
                    else:
                        nc.vector.tensor_mul(
                            wv[:, (ci - g0) * P:(ci - g0 + nn) * P]
                            .rearrange("p (c h d) -> p c h d", h=H, d=Dh),
                            vt[:, (cc - t0) * P:(cc - t0 + nn) * P]
                            .rearrange("p (c h d) -> p c h d", h=H, d=Dh),
                            wsc[:, ci * H:(ci + nn) * H]
                            .rearrange("p (c h) -> p c h", h=H)
                            [:, :, :, None]
                            .broadcast_to([P, nn, H, Dh]))
                    ci = cj
                wvs[i] = wv

            def stage_m(i):
                b, g0, gn, cs, ce = groups[i]
                wv = wvs.pop(i)
                if g0 == cs:
                    aggps[b] = apsum.tile([P, P], F32, tag="agg", name="aggp")
                aggp = aggps[b]
                for ci in range(g0, g0 + gn):
                    tt, t0, _ = stream(t_tiles, tpool, TT_st, ci, F8)
                    nc.tensor.matmul(
                        aggp[:],
                        lhsT=wv[:, (ci - g0) * P:(ci - g0 + 1) * P],
                        rhs=tt[:, (ci - t0) * P:(ci - t0 + 1) * P],
                        start=(ci == cs), stop=(ci == ce - 1))
                if g0 + gn != ce:
                    return
                aggp = aggps.pop(b)
                agg16 = wpool.tile([P, P], F16, tag="agg16", name="agg16")
                nc.scalar.copy(agg16[:], aggp[:])
                outp = opsum.tile([P, P], F32, tag="outp", name="outp")
                nc.tensor.matmul(outp[:], lhsT=wo_sb[:], rhs=agg16[:],
                                 start=True, stop=False)
                nc.tensor.matmul(outp[:], lhsT=bo_sb[:], rhs=ones_sb[:],
                                 start=False, stop=True)
                nc.scalar.copy(osb[:, b * P:(b + 1) * P], outp[:])
                if b % DB == DB - 1 or b == NB - 1:
                    a0 = (b // DB) * DB * P
                    a1 = (b + 1) * P
                    nc.gpsimd.dma_start(outT[:, a0:a1], osb[:, a0:a1])

            for i in range(ng + 1):
                if i < ng:
                    stage_v(i)
                if 1 <= i <= ng:
                    stage_m(i - 1)
    nc.compile()
    return nc


# ---------------------------------------------------------------- orchestration
def _prep_weights(Wq, bq, Wk, bk, Wv, bv, Wo, bo):
    w16 = {k: np.asarray(v, np.float32).astype(np.float16)
           for k, v in (("Wq", Wq), ("Wk", Wk), ("Wv", Wv), ("Wo", Wo))}
    b16 = {k: np.asarray(v, np.float32).astype(np.float16)
           for k, v in (("bq", bq), ("bk", bk), ("bv", bv), ("bo", bo))}
    return w16, b16


def kernel(node_features, edge_index, Wq, bq, Wk, bk, Wv, bv, Wo, bo):
    node_features = np.asarray(node_features, np.float32)
    edge_index = np.asarray(edge_index)
    src, dst = edge_index[0].astype(np.int64), edge_index[1].astype(np.int64)
    x16 = node_features.astype(np.float16)
    w16, b16 = _prep_weights(Wq, bq, Wk, bk, Wv, bv, Wo, bo)
    ones_row = np.ones((1, P), np.float16)
    cores = list(range(C))
    eids = np.arange(E, dtype=np.int64)

    # ---------------- L1
    nc1 = build_l1()
    in1 = []
    for c in cores:
        base, ln = shard_base(c), shard_len(c)
        xt = np.zeros((P, NB * P), np.float16)
        xt[:, :ln] = x16[base:base + ln].T
        in1.append(dict(
            xT=xt,
            wqkv=np.concatenate([w16["Wq"].T, w16["Wk"].T, w16["Wv"].T],
                                axis=1).copy(),
            bqkv=np.concatenate([b16["bq"], b16["bk"], b16["bv"]])
            .reshape(1, 3 * P), ones=ones_row))
    r1 = run_bass_kernel_spmd(nc1, in1, core_ids=cores)

    q_shs = []
    k_all = np.zeros((N, P), np.float16)
    v_all = np.zeros((N, P), np.float16)
    for c in cores:
        base, ln = shard_base(c), shard_len(c)
        blob = r1.results[c]["qkv"].reshape(P, NB, 3, P)
        q_shs.append(np.ascontiguousarray(blob[:, :, 0, :].reshape(P, NB * P)))
        k_sh = blob[:, :, 1, :].transpose(1, 0, 2).reshape(NB * P, P)
        v_sh = blob[:, :, 2, :].transpose(1, 0, 2).reshape(NB * P, P)
        k_all[base:base + ln] = k_sh[:ln]
        v_all[base:base + ln] = v_sh[:ln]

    k8, krs = row_quant_int8(k_all)
    v8, vrs = row_quant_int8(v_all)

    # ---------------- L2
    cmap2 = compute_cmap(src)
    plans2 = []
    for c in cores:
        base, ln = shard_base(c), shard_len(c)
        m = (src >= base) & (src < base + ln)
        plans2.append(CorePlan(cmap2, c, src[m], dst[m], eids[m]))

    nc2 = build_l2(cmap2)
    Bmat = head_mask_matrix()
    kf16, koff, kn8, kn16 = chunk_split(cmap2, *L2_SPLIT, tail=True)
    in2 = []
    for c in cores:
        pl = plans2[c]
        nch = cmap2.nch
        oth = pl.slot_other.reshape(nch, P)
        k8_rows = k8[oth[~kf16]]                       # [n8, P, P] int8
        k16_rows = k_all[oth[kf16]]                    # [n16, P, P] f16
        kst8 = k8_rows.transpose(2, 0, 1).reshape(P, -1).copy() \
            if kn8 else np.zeros((P, P), np.int8)
        kst16 = k16_rows.transpose(2, 0, 1).reshape(P, -1).copy() \
            if kn16 else np.zeros((P, P), np.float16)
        valid = (pl.slot_edge >= 0).astype(np.float32)
        scale = np.where(np.repeat(kf16, P), 1.0, krs[pl.slot_other])
        srow_v = (scale * valid * 0.25).astype(np.float16)
        in2.append(dict(
            q_sh=q_shs[c], kst8=kst8, kst16=kst16,
            srow=np.ascontiguousarray(srow_v.reshape(nch, P).T),
            S_st=pl.onehot_stream(False), ST_st=pl.onehot_stream(True),
            Bm=Bmat))
    r2 = run_bass_kernel_spmd(nc2, in2, core_ids=cores)

    exp_edge = np.zeros((E, H), np.float16)
    rec_all = np.zeros((N, H), np.float16)
    for c in cores:
        pl = plans2[c]
        e_flat = r2.results[c]["exp_out"].reshape(P, cmap2.nch, H) \
            .transpose(1, 0, 2).reshape(cmap2.nslots, H)
        real = pl.slot_edge >= 0
        exp_edge[pl.slot_edge[real]] = e_flat[real]
        base, ln = shard_base(c), shard_len(c)
        rec_sh = r2.results[c]["rec_out"].reshape(P, NB, H) \
            .transpose(1, 0, 2).reshape(NB * P, H)
        rec_all[base:base + ln] = rec_sh[:ln]

    # ---------------- L3
    cmap3 = compute_cmap(dst)
    plans3 = []
    for c in cores:
        base, ln = shard_base(c), shard_len(c)
        m = (dst >= base) & (dst < base + ln)
        plans3.append(CorePlan(cmap3, c, dst[m], src[m], eids[m]))

    nc3 = build_l3(cmap3)
    vf16, voff, vn8, vn16 = chunk_split(cmap3, *L3_SPLIT)
    in3 = []
    for c in cores:
        pl = plans3[c]
        nch = cmap3.nch
        oth = pl.slot_other.reshape(nch, P)
        v8_rows = v8[oth[~vf16]]
        v16_rows = v_all[oth[vf16]]
        vst8 = v8_rows.transpose(1, 0, 2).reshape(P, -1).copy() \
            if vn8 else np.zeros((P, P), np.int8)
        vst16 = v16_rows.transpose(1, 0, 2).reshape(P, -1).copy() \
            if vn16 else np.zeros((P, P), np.float16)
        e_slots = np.zeros((cmap3.nslots, H), np.float16)
        real = pl.slot_edge >= 0
        e_slots[real] = exp_edge[pl.slot_edge[real]]
        # combined per-slot scale: softmax denominator recip at the src node
        # times the src v-row int8 scale (1 for f16 chunks, 0 on padding)
        vscale = np.where(np.repeat(vf16, P), 1.0, vrs[pl.slot_other])
        srw_v = (rec_all[pl.slot_other].astype(np.float32) *
                 (vscale * real.astype(np.float32))[:, None]) \
            .astype(np.float16)
        in3.append(dict(
            vst8=vst8, vst16=vst16,
            exp_in=np.ascontiguousarray(
                e_slots.reshape(nch, P, H).transpose(1, 0, 2)
                .reshape(P, nch * H)),
            srw=np.ascontiguousarray(
                srw_v.reshape(nch, P, H).transpose(1, 0, 2)
                .reshape(P, nch * H)),
            TT_st=pl.onehot_stream(True),
            WoT=w16["Wo"].T.copy(),
            bo_r=b16["bo"].reshape(1, P), ones=ones_row))
    r3 = run_bass_kernel_spmd(nc3, in3, core_ids=cores)

    out = np.zeros((N, F), np.float32)
    for c in cores:
        base, ln = shard_base(c), shard_len(c)
        o = r3.results[c]["outT"].reshape(P, NB, P).transpose(1, 2, 0) \
            .reshape(NB * P, P)
        out[base:base + ln] = o[:ln].astype(np.float32)
    return out


# revision 52
# speedup vs baseline: 1.1064x; 1.0229x over previous
"""Trainium2 Bass kernel for nn_EnhancedReflectiveCognitiveGraph (GNN edge-softmax attention).

Math (see reference):
  q/k/v = x @ W{q,k,v}.T + b ; per-edge scores s_e = <q[src_e], k[dest_e]>_head / 4
  softmax over edges sharing src (max-subtraction skipped: scores ~ N(0,1) so
  exp never overflows and the weights are mathematically identical)
  agg[dest] += w_e * v[src_e] ; out = agg @ Wo.T + bo

Device strategy (8 cores, node-range sharding, three SPMD launches):
  L1 (proj): each core computes q/k/v (fp16) for its node shard.
  L2 (src phase): core c owns edges with src in its shard, laid out in
      128-edge chunks grouped by 128-node src block.  The k rows for each
      edge slot arrive as a host-prepared per-slot int8 stream (contiguous,
      full DMA bandwidth; per-row quantization scales are applied to the
      reduced scores, not the rows).  q rows are expanded per-edge on-chip
      via PE matmuls against streamed one-hot matrices in [feat x slot]
      orientation; the per-head dot products are then a second PE matmul
      against a tiny constant block-diagonal matrix, so no DVE reduction is
      needed.  exp -> per-src-block segment sums via PE matmuls with
      one-hots -> reciprocal -> per-edge softmax weights w_e (output).
  L3 (dest phase): core c owns edges with dest in its shard.  v rows arrive
      as a per-slot int8 stream; weighted rows (w_e * v) are scatter-added
      into per-dest-block agg via PE matmuls with one-hots, then the output
      projection.  No collectives and no device-side gathers anywhere.
  Host between launches does relayout only: assembling tables from L1/L2
  outputs, per-row int8 packing, per-slot stream/one-hot construction, and
  permutation of edge weights between the src- and dest-groupings.
"""

import math
import ml_dtypes
import numpy as np

import concourse.bacc as bacc
import concourse.mybir as mybir
import concourse.tile as tile
from concourse.bass_utils import run_bass_kernel_spmd

# ---------------------------------------------------------------- constants
N = 50000
E = 600000
F = 128
H = 8
Dh = 16
P = 128
C = 8                     # cores
SH = 6272                 # nodes per core, cores 0-6 (49 blocks); core 7: 6096
NB = 49                   # blocks per shard
G = 8                     # chunks per processing group (psum-sized)
KB = 32                   # chunks per stream DMA tile (L2)
KB3 = 64                  # chunks per stream DMA tile (L3)
SG = 12                   # blocks per recip supergroup in L2
F16 = mybir.dt.float16
F8 = mybir.dt.float8e4
F32 = mybir.dt.float32
I8 = mybir.dt.int8


def shard_base(c):
    return c * SH


def shard_len(c):
    return min(N, (c + 1) * SH) - c * SH


# ---------------------------------------------------------------- host prep
class ChunkMap:
    """Uniform chunk structure shared by all cores for one phase.

    Chunks (128 slots each) are block-major: kb[b] chunks for block b; the
    chunk->block map is identical on every core so one program serves all 8."""

    def __init__(self, kb):
        self.kb = [int(x) for x in kb]
        self.chunks = [b for b in range(NB) for _ in range(self.kb[b])]
        self.nch = len(self.chunks)
        self.nslots = self.nch * P
        self.start = np.concatenate([[0], np.cumsum(self.kb)]).astype(int)


def compute_cmap(key, other=None):
    """Global uniform per-block chunk counts for one phase."""
    kb = np.ones(NB, np.int64)
    for c in range(C):
        base, ln = shard_base(c), shard_len(c)
        m = (key >= base) & (key < base + ln)
        cnt = np.bincount((key[m] - base) // P, minlength=NB)
        kb = np.maximum(kb, (cnt + P - 1) // P)
    return ChunkMap(kb)


def group_list(cmap):
    """Processing groups of up to G chunks, block-aligned: (b, g0, gn, cs, ce)."""
    groups = []
    for b in range(NB):
        cs, ce = int(cmap.start[b]), int(cmap.start[b + 1])
        for g0 in range(cs, ce, G):
            groups.append((b, g0, min(G, ce - g0), cs, ce))
    return groups


def chunk_split(cmap, num, den, tail=False, shift=0):
    """Mixed-precision chunk routing: `num` of every `den` groups take the
    f16 path (Act-assisted 2x multiply), the rest the int8 path (half DMA).
    `tail` places the f16 groups at the end of each cycle (keeps the large
    f16 stream tiles off the pipeline ramp).
    Returns (f16 flag per chunk, compact offset per chunk, n8, n16)."""
    f16 = np.zeros(cmap.nch, bool)
    for i, (b, g0, gn, cs, ce) in enumerate(group_list(cmap)):
        sel = (i % den >= den - num) if tail else ((i - shift) % den < num)
        if sel:
            f16[g0:g0 + gn] = True
    off = np.zeros(cmap.nch, np.int64)
    n8 = n16 = 0
    for ci in range(cmap.nch):
        if f16[ci]:
            off[ci] = n16
            n16 += 1
        else:
            off[ci] = n8
            n8 += 1
    return f16, off, n8, n16


class CorePlan:
    """Per-core slot contents for one phase.  `key` = node defining the block
    (src for L2, dest for L3); `other` = node whose row the slot consumes."""

    def __init__(self, cmap, core, key, other, edge_ids):
        base = shard_base(core)
        nsl = cmap.nslots
        self.slot_local = np.full(nsl, -1, np.int64)
        self.slot_other = np.zeros(nsl, np.int64)
        self.slot_edge = np.full(nsl, -1, np.int64)
        block = (key - base) // P
        for b in range(NB):
            m = block == b
            cnt = int(m.sum())
            if cnt == 0:
                continue
            assert cnt <= cmap.kb[b] * P
            s0 = int(cmap.start[b]) * P
            self.slot_local[s0:s0 + cnt] = key[m] - base - b * P
            self.slot_other[s0:s0 + cnt] = other[m]
            self.slot_edge[s0:s0 + cnt] = edge_ids[m]
        self.cmap = cmap

    def onehot_stream(self, transposed):
        """[128, nch*128] fp8; chunk c at cols c*128:(c+1)*128.
        transposed=False: S [loc, (c,slot)] ; True: ST/TT [slot, (c,loc)].
        Dummy slots are all-zero columns/rows."""
        cm = self.cmap
        out = np.zeros((P, cm.nch * P), dtype=ml_dtypes.float8_e4m3)
        loc = self.slot_local
        sl_all = np.arange(cm.nslots)
        valid = loc >= 0
        ch = sl_all // P
        row = sl_all % P
        if transposed:
            out[row[valid], ch[valid] * P + loc[valid]] = 1.0
        else:
            out[loc[valid], ch[valid] * P + row[valid]] = 1.0
        return out


def row_quant_int8(a16):
    """Per-row symmetric int8 quantization.  Returns (int8 vals, f32 scales)."""
    a = np.asarray(a16, np.float32)
    am = np.abs(a).max(axis=1)
    s = np.where(am > 0, am / 127.0, 1.0).astype(np.float32)
    q = np.clip(np.round(a / s[:, None]), -127, 127).astype(np.int8)
    return q, s


def head_mask_matrix():
    """[128, 8] fp8 block-diagonal ones: B[f, h] = (f // 16 == h)."""
    B = np.zeros((P, H), dtype=ml_dtypes.float8_e4m3)
    for h in range(H):
        B[h * Dh:(h + 1) * Dh, h] = 1.0
    return B


# ---------------------------------------------------------------- L1: projections
def build_l1(with_bias=False):
    nc = bacc.Bacc("TRN2", target_bir_lowering=False, num_devices=C)
    xT = nc.dram_tensor("xT", [P, NB * P], F16, kind="ExternalInput")
    wqkv = nc.dram_tensor("wqkv", [P, 3 * P], F16, kind="ExternalInput")
    bqkv = nc.dram_tensor("bqkv", [1, 3 * P], F16, kind="ExternalInput")
    ones = nc.dram_tensor("ones", [1, P], F16, kind="ExternalInput")
    qkv = nc.dram_tensor("qkv", [P, NB * 3 * P], F16, kind="ExternalOutput")

    with tile.TileContext(nc) as tc:
        with tc.tile_pool(name="const", bufs=1) as cpool, \
             tc.tile_pool(name="stage", bufs=3) as spool, \
             tc.tile_pool(name="psum", bufs=4, space="PSUM") as ppool:
            w_sb = cpool.tile([P, 3 * P], F16, tag="w", name="w_sb")
            nc.sync.dma_start(w_sb[:], wqkv[:])
            b_sb = cpool.tile([1, 3 * P], F16, tag="b", name="b_sb")
            nc.sync.dma_start(b_sb[:], bqkv[:])
            ones_sb = cpool.tile([1, P], F16, tag="ones", name="ones_sb")
            nc.sync.dma_start(ones_sb[:], ones[:])
            xt = cpool.tile([P, NB * P], F16, tag="xT", name="xt")
            for i in range(4):
                a = i * 13 * P
                b = min(NB, (i + 1) * 13) * P
                nc.sync.dma_start(xt[:, a:b], xT[:, a:b])
            DB = 7   # blocks per output DMA
            osb = cpool.tile([P, NB * 3 * P], F16, tag="osb", name="osb")
            for b in range(NB):
                ps = ppool.tile([P, 3 * P], F32, tag="proj", name="ps")
                if with_bias:
                    nc.tensor.matmul(ps[:], lhsT=xt[:, b * P:(b + 1) * P],
                                     rhs=w_sb[:], start=True, stop=False)
                    nc.tensor.matmul(ps[:], lhsT=ones_sb[:], rhs=b_sb[:],
                                     start=False, stop=True)
                else:
                    nc.tensor.matmul(ps[:], lhsT=xt[:, b * P:(b + 1) * P],
                                     rhs=w_sb[:], start=True, stop=True)
                dstsl = osb[:, b * 3 * P:(b + 1) * 3 * P]
                if b % 2 == 0:
                    nc.vector.tensor_copy(dstsl, ps[:])
                else:
                    nc.scalar.copy(dstsl, ps[:])
                if b % DB == DB - 1 or b == NB - 1:
                    a0 = (b // DB) * DB * 3 * P
                    a1 = (b + 1) * 3 * P
                    nc.gpsimd.dma_start(qkv[:, a0:a1], osb[:, a0:a1])
    nc.compile()
    return nc


# ---------------------------------------------------------------- L2: src phase
L2_SPLIT = (1, 4)   # 1/4 of groups take the f16 k path


def build_l2(cmap):
    nch = cmap.nch
    kf16, koff, n8, n16 = chunk_split(cmap, *L2_SPLIT, tail=True)
    nc = bacc.Bacc("TRN2", target_bir_lowering=False, num_devices=C)
    q_sh = nc.dram_tensor("q_sh", [P, NB * P], F16, kind="ExternalInput")
    kst8 = nc.dram_tensor("kst8", [P, max(n8, 1) * P], I8, kind="ExternalInput")
    kst16 = nc.dram_tensor("kst16", [P, max(n16, 1) * P], F16,
                           kind="ExternalInput")
    srow = nc.dram_tensor("srow", [P, nch], F16, kind="ExternalInput")
    S_st = nc.dram_tensor("S_st", [P, nch * P], F8, kind="ExternalInput")
    ST_st = nc.dram_tensor("ST_st", [P, nch * P], F8, kind="ExternalInput")
    Bm = nc.dram_tensor("Bm", [P, H], F8, kind="ExternalInput")
    exp_out = nc.dram_tensor("exp_out", [P, nch * H], F16, kind="ExternalOutput")
    rec_out = nc.dram_tensor("rec_out", [P, NB * H], F16, kind="ExternalOutput")

    groups = group_list(cmap)
    ng = len(groups)

    with tile.TileContext(nc) as tc:
        with tc.tile_pool(name="res", bufs=1) as rpool, \
             tc.tile_pool(name="kst", bufs=3) as kpool, \
             tc.tile_pool(name="st", bufs=3) as tpool, \
             tc.tile_pool(name="sst", bufs=3) as spool, \
             tc.tile_pool(name="work", bufs=6) as wpool, \
             tc.tile_pool(name="qpsum", bufs=3, space="PSUM") as qpsum, \
             tc.tile_pool(name="bpsum", bufs=2, space="PSUM") as bpsum:
            B_sb = rpool.tile([P, H], F8, tag="B", name="B_sb")
            nc.sync.dma_start(B_sb[:], Bm[:])
            srow_sb = rpool.tile([P, nch], F16, tag="srow", name="srow_sb")
            nc.sync.dma_start(srow_sb[:], srow[:])
            q_sb = rpool.tile([P, NB * P], F16, tag="q", name="q_sb")
            q_spans = set()

            def want_q(b):
                i = b * 4 // NB
                if i not in q_spans:
                    q_spans.add(i)
                    a = (i * NB + 3) // 4 * P
                    bb = ((i + 1) * NB + 3) // 4 * P
                    nc.sync.dma_start(q_sb[:, a:bb], q_sh[:, a:bb])
            exp_sb = rpool.tile([P, nch * H], F16, tag="exp", name="exp_sb")
            seg_sb = rpool.tile([P, NB * H], F32, tag="seg", name="seg_sb")
            rec_sb = rpool.tile([P, NB * H], F16, tag="rec", name="rec_sb")

            k_tiles = {}
            k16_tiles = {}
            t_tiles = {}
            s_tiles = {}

            def stream(tiles, pool, dram, ci, width=P, dt=I8, total=None,
                       off=None):
                cc = ci if off is None else int(off[ci])
                tot = nch if total is None else total
                t0 = cc // KB * KB
                if t0 not in tiles:
                    t = pool.tile([P, KB * width], dt, tag=dram.name,
                                  name=f"strm_{dram.name}_{t0}")
                    n = min(KB, tot - t0) * width
                    nc.sync.dma_start(t[:, :n], dram[:, t0 * width:t0 * width + n])
                    tiles[t0] = t
                return tiles[t0], t0, cc

            # Software-pipelined stages, skewed so PE never queue-stalls on a
            # cross-engine dependency:
            #   A(i):   q expansion (PE) + qk multiply (DVE)
            #   B(i-1): score matmuls (PE)
            #   C(i-2): on last group of a block: dequant (DVE), exp (Act),
            #           segment-sum matmuls (PE), seg copy (Act)
            qkTs = {}
            scps = {}

            def prefetch(i):
                # touch the stream spans a group ahead so span-boundary DMA
                # latency never stalls the compute pipeline
                b, g0, gn, cs, ce = groups[i]
                ce2 = min(g0 + gn + G, nch) - 1
                stream(s_tiles, spool, S_st, ce2, dt=F8)
                if kf16[ce2]:
                    stream(k16_tiles, kpool, kst16, ce2, dt=F16, total=n16,
                           off=koff)
                else:
                    stream(k_tiles, kpool, kst8, ce2, dt=I8, total=n8,
                           off=koff)
                stream(t_tiles, tpool, ST_st, ce2, dt=F8)

            def stage_a(i):
                b, g0, gn, cs, ce = groups[i]
                want_q(b)
                want_q(min(b + 3, NB - 1))
                qeT = qpsum.tile([P, G * P], F32, tag="qeT", name="qeT")
                ci = g0
                while ci < g0 + gn:
                    st, t0, _ = stream(s_tiles, spool, S_st, ci, dt=F8)
                    # pieces must not cross 512-col psum bank boundaries
                    cj = min(g0 + gn, t0 + KB, g0 + ((ci - g0) // 4 + 1) * 4)
                    nsl = (cj - ci) * P
                    nc.tensor.matmul(
                        qeT[:, (ci - g0) * P:(ci - g0) * P + nsl],
                        lhsT=q_sb[:, b * P:(b + 1) * P],
                        rhs=st[:, (ci - t0) * P:(ci - t0) * P + nsl],
                        start=True, stop=True)
                    ci = cj
                qkT = wpool.tile([P, G * P], F16, tag="qkT", name="qkT")
                f16p = bool(kf16[g0])
                if f16p:
                    qe16 = wpool.tile([P, G * P], F16, tag="qe16", name="qe16")
                    nc.scalar.copy(qe16[:, :gn * P], qeT[:, :gn * P])
                    in0 = qe16
                else:
                    in0 = qeT
                ci = g0
                while ci < g0 + gn:
                    if f16p:
                        kt, t0, cc = stream(k16_tiles, kpool, kst16, ci,
                                            dt=F16, total=n16, off=koff)
                    else:
                        kt, t0, cc = stream(k_tiles, kpool, kst8, ci,
                                            dt=I8, total=n8, off=koff)
                    cj = min(g0 + gn, ci + (t0 + KB - cc))
                    nsl = (cj - ci) * P
                    nc.vector.tensor_mul(
                        qkT[:, (ci - g0) * P:(ci - g0) * P + nsl],
                        in0[:, (ci - g0) * P:(ci - g0) * P + nsl],
                        kt[:, (cc - t0) * P:(cc - t0) * P + nsl])
                    ci = cj
                qkTs[i] = qkT

            def stage_b(i):
                b, g0, gn, cs, ce = groups[i]
                qkT = qkTs.pop(i)
                if g0 == cs:
                    scps[b] = bpsum.tile([P, 17 * H], F32, tag="blk",
                                         name="blkps")
                scp = scps[b]
                for ci in range(g0, g0 + gn):
                    nc.tensor.matmul(
                        scp[:, (ci - cs) * H:(ci - cs + 1) * H],
                        lhsT=qkT[:, (ci - g0) * P:(ci - g0 + 1) * P],
                        rhs=B_sb[:], start=True, stop=True)

            def stage_c(i):
                b, g0, gn, cs, ce = groups[i]
                if g0 + gn != ce:
                    return
                nb = ce - cs
                blkps = scps.pop(b)
                scp = blkps
                sc16 = wpool.tile([P, 16 * H], F16, tag="sc16", name="sc16")
                nc.vector.tensor_mul(
                    sc16[:, :nb * H].rearrange("p (c h) -> p c h", h=H),
                    scp[:, :nb * H].rearrange("p (c h) -> p c h", h=H),
                    srow_sb[:, cs:ce][:, :, None]
                    .broadcast_to([P, nb, H]))
                nc.scalar.activation(
                    out=exp_sb[:, cs * H:ce * H],
                    in_=sc16[:, :nb * H],
                    func=mybir.ActivationFunctionType.Exp,
                    scale=1.0)
                segp = blkps[:, 16 * H:17 * H]
                for ci in range(cs, ce):
                    tt, t0, _ = stream(t_tiles, tpool, ST_st, ci, dt=F8)
                    nc.tensor.matmul(
                        segp,
                        lhsT=tt[:, (ci - t0) * P:(ci - t0 + 1) * P],
                        rhs=exp_sb[:, ci * H:(ci + 1) * H],
                        start=(ci == cs), stop=(ci == ce - 1))
                nc.scalar.copy(seg_sb[:, b * H:(b + 1) * H], segp)

            for i in range(ng + 2):
                if i < ng:
                    stage_a(i)
                    prefetch(i)
                if 1 <= i <= ng:
                    stage_b(i - 1)
                if 2 <= i <= ng + 1:
                    stage_c(i - 2)
                    bdone = groups[i - 2][0]
                    if groups[i - 2][1] + groups[i - 2][2] == groups[i - 2][4]:
                        # exp_out slice per ~8 finished blocks
                        if bdone % 8 == 7 or bdone == NB - 1:
                            a = int(cmap.start[bdone // 8 * 8]) * H
                            bcol = int(cmap.start[bdone + 1]) * H
                            nc.gpsimd.dma_start(exp_out[:, a:bcol],
                                                exp_sb[:, a:bcol])

            # reciprocal; empty segments (zero-degree locs, padding) get
            # seg+1 so it stays finite — their one-hot columns are all-zero
            # downstream so the value never contributes.
            seg1 = wpool.tile([P, NB * H], F32, tag="seg1", name="seg1")
            nc.vector.scalar_tensor_tensor(
                out=seg1[:], in0=seg_sb[:], scalar=0.0, in1=seg_sb[:],
                op0=mybir.AluOpType.is_le, op1=mybir.AluOpType.add)
            with nc.allow_low_precision(reason="softmax recip fits f16"):
                nc.vector.reciprocal(rec_sb[:], seg1[:])
            nc.gpsimd.dma_start(rec_out[:], rec_sb[:])
    nc.compile()
    return nc


# ---------------------------------------------------------------- L3: dest phase
L3_SPLIT = (2, 5)   # 2/5 of groups take the f16 v path


def build_l3(cmap, with_bias=False):
    nch = cmap.nch
    vf16, voff, n8, n16 = chunk_split(cmap, *L3_SPLIT)
    nc = bacc.Bacc("TRN2", target_bir_lowering=False, num_devices=C)
    vst8 = nc.dram_tensor("vst8", [P, max(n8, 1) * P], I8, kind="ExternalInput")
    vst16 = nc.dram_tensor("vst16", [P, max(n16, 1) * P], F16,
                           kind="ExternalInput")
    exp_in = nc.dram_tensor("exp_in", [P, nch * H], F16, kind="ExternalInput")
    srw = nc.dram_tensor("srw", [P, nch * H], F16, kind="ExternalInput")
    TT_st = nc.dram_tensor("TT_st", [P, nch * P], F8, kind="ExternalInput")
    WoT = nc.dram_tensor("WoT", [P, P], F16, kind="ExternalInput")
    bo_r = nc.dram_tensor("bo_r", [1, P], F16, kind="ExternalInput")
    ones = nc.dram_tensor("ones", [1, P], F16, kind="ExternalInput")
    outT = nc.dram_tensor("outT", [P, NB * P], F16, kind="ExternalOutput")

    with tile.TileContext(nc) as tc:
        with tc.tile_pool(name="res", bufs=1) as rpool, \
             tc.tile_pool(name="vstp", bufs=2) as vpool, \
             tc.tile_pool(name="tt", bufs=2) as tpool, \
             tc.tile_pool(name="work", bufs=3) as wpool, \
             tc.tile_pool(name="apsum", bufs=2, space="PSUM") as apsum, \
             tc.tile_pool(name="opsum", bufs=2, space="PSUM") as opsum:
            wo_sb = rpool.tile([P, P], F16, tag="wo", name="wo_sb")
            nc.sync.dma_start(wo_sb[:], WoT[:])
            bo_sb = rpool.tile([1, P], F16, tag="bo", name="bo_sb")
            nc.sync.dma_start(bo_sb[:], bo_r[:])
            ones_sb = rpool.tile([1, P], F16, tag="ones", name="ones_sb")
            nc.sync.dma_start(ones_sb[:], ones[:])
            osb = rpool.tile([P, NB * P], F16, tag="osb", name="osb")
            w_sb = rpool.tile([P, nch * H], F16, tag="w", name="w_sb")
            sv_sb = rpool.tile([P, nch * H], F16, tag="sv", name="sv_sb")
            wsc = rpool.tile([P, nch * H], F16, tag="wsc", name="wsc")

            v_tiles = {}
            v16_tiles = {}
            t_tiles = {}
            w_spans = set()

            def stream(tiles, pool, dram, ci, dt, total=None, off=None):
                cc = ci if off is None else int(off[ci])
                tot = nch if total is None else total
                t0 = cc // KB3 * KB3
                if t0 not in tiles:
                    t = pool.tile([P, KB3 * P], dt, tag=dram.name,
                                  name=f"strm_{dram.name}_{t0}")
                    n = min(KB3, tot - t0) * P
                    nc.sync.dma_start(t[:, :n], dram[:, t0 * P:t0 * P + n])
                    tiles[t0] = t
                return tiles[t0], t0, cc

            def want_wsc(ci):
                # per-edge weight exp * (rec[src] * v-row-scale): lazily DMA'd
                # and computed (packed f16 2x mult) per KB3-chunk span
                t0 = ci // KB3 * KB3
                if t0 not in w_spans:
                    w_spans.add(t0)
                    a = t0 * H
                    bcol = min(nch, t0 + KB3) * H
                    nc.sync.dma_start(w_sb[:, a:bcol], exp_in[:, a:bcol])
                    nc.sync.dma_start(sv_sb[:, a:bcol], srw[:, a:bcol])
                    nc.vector.tensor_mul(wsc[:, a:bcol], w_sb[:, a:bcol],
                                         sv_sb[:, a:bcol])

            DB = 7
            groups = group_list(cmap)
            ng = len(groups)
            wvs = {}
            aggps = {}

            def stage_v(i):
                b, g0, gn, cs, ce = groups[i]
                wv = wpool.tile([P, G * P], F16, tag="wv", name="wv")
                want_wsc(g0)
                want_wsc(g0 + gn - 1)
                f16p = bool(vf16[g0])
                erep = None
                if f16p:
                    # materialize the broadcast weights on Act so the
                    # multiply runs packed-f16 at DVE 2x
                    erep = wpool.tile([P, G * P], F16, tag="erep",
                                      name="erep")
                    nc.scalar.copy(
                        erep[:, :gn * P]
                        .rearrange("p (c h d) -> p c h d", h=H, d=Dh),
                        wsc[:, g0 * H:(g0 + gn) * H]
                        .rearrange("p (c h) -> p c h", h=H)[:, :, :, None]
                        .broadcast_to([P, gn, H, Dh]))
                ci = g0
                while ci < g0 + gn:
                    if f16p:
                        vt, t0, cc = stream(v16_tiles, vpool, vst16, ci,
                                            F16, total=n16, off=voff)
                    else:
                        vt, t0, cc = stream(v_tiles, vpool, vst8, ci,
                                            I8, total=n8, off=voff)
                    cj = min(g0 + gn, ci + (t0 + KB3 - cc))
                    nn = cj - ci
                    if f16p:
                        nc.vector.tensor_mul(
                            wv[:, (ci - g0) * P:(ci - g0 + nn) * P],
                            vt[:, (cc - t0) * P:(cc - t0 + nn) * P],
                            erep[:, (ci - g0) * P:(ci - g0 + nn) * P])
                    else:
                        nc.vector.tensor_mul(
                            wv[:, (ci - g0) * P:(ci - g0 + nn) * P]
                            .rearrange("p (c h d) -> p c h d", h=H, d=Dh),
                            vt[:, (cc - t0) * P:(cc - t0 + nn) * P]
                            .rearrange("p (c h d) -> p c h d", h=H, d=Dh),
                            wsc[:, ci * H:(ci + nn) * H]
                            .rearrange("p (c h) -> p c h", h=H)
                            [:, :, :, None]
                            .broadcast_to([P, nn, H, Dh]))
                    ci = cj
                wvs[i] = wv

            def stage_m(i):
                b, g0, gn, cs, ce = groups[i]
                wv = wvs.pop(i)
                if g0 == cs:
                    aggps[b] = apsum.tile([P, P], F32, tag="agg", name="aggp")
                aggp = aggps[b]
                for ci in range(g0, g0 + gn):
                    tt, t0, _ = stream(t_tiles, tpool, TT_st, ci, F8)
                    nc.tensor.matmul(
                        aggp[:],
                        lhsT=wv[:, (ci - g0) * P:(ci - g0 + 1) * P],
                        rhs=tt[:, (ci - t0) * P:(ci - t0 + 1) * P],
                        start=(ci == cs), stop=(ci == ce - 1))
                if g0 + gn != ce:
                    return
                aggp = aggps.pop(b)
                agg16 = wpool.tile([P, P], F16, tag="agg16", name="agg16")
                nc.scalar.copy(agg16[:], aggp[:])
                outp = opsum.tile([P, P], F32, tag="outp", name="outp")
                if with_bias:
                    nc.tensor.matmul(outp[:], lhsT=wo_sb[:], rhs=agg16[:],
                                     start=True, stop=False)
                    nc.tensor.matmul(outp[:], lhsT=bo_sb[:], rhs=ones_sb[:],
                                     start=False, stop=True)
                else:
                    nc.tensor.matmul(outp[:], lhsT=wo_sb[:], rhs=agg16[:],
                                     start=True, stop=True)
                nc.scalar.copy(osb[:, b * P:(b + 1) * P], outp[:])
                if b % DB == DB - 1 or b == NB - 1:
                    a0 = (b // DB) * DB * P
                    a1 = (b + 1) * P
                    nc.gpsimd.dma_start(outT[:, a0:a1], osb[:, a0:a1])

            for i in range(ng + 1):
                if i < ng:
                    stage_v(i)
                if 1 <= i <= ng:
                    stage_m(i - 1)
    nc.compile()
    return nc


# ---------------------------------------------------------------- orchestration
def _prep_weights(Wq, bq, Wk, bk, Wv, bv, Wo, bo):
    w16 = {k: np.asarray(v, np.float32).astype(np.float16)
           for k, v in (("Wq", Wq), ("Wk", Wk), ("Wv", Wv), ("Wo", Wo))}
    b16 = {k: np.asarray(v, np.float32).astype(np.float16)
           for k, v in (("bq", bq), ("bk", bk), ("bv", bv), ("bo", bo))}
    return w16, b16


def kernel(node_features, edge_index, Wq, bq, Wk, bk, Wv, bv, Wo, bo):
    node_features = np.asarray(node_features, np.float32)
    edge_index = np.asarray(edge_index)
    src, dst = edge_index[0].astype(np.int64), edge_index[1].astype(np.int64)
    x16 = node_features.astype(np.float16)
    w16, b16 = _prep_weights(Wq, bq, Wk, bk, Wv, bv, Wo, bo)
    ones_row = np.ones((1, P), np.float16)
    cores = list(range(C))
    eids = np.arange(E, dtype=np.int64)

    any_bias_in = any(np.any(np.asarray(b) != 0) for b in (bq, bk, bv))
    any_bias_out = bool(np.any(np.asarray(bo) != 0))

    # ---------------- L1
    nc1 = build_l1(with_bias=any_bias_in)
    in1 = []
    for c in cores:
        base, ln = shard_base(c), shard_len(c)
        xt = np.zeros((P, NB * P), np.float16)
        xt[:, :ln] = x16[base:base + ln].T
        in1.append(dict(
            xT=xt,
            wqkv=np.concatenate([w16["Wq"].T, w16["Wk"].T, w16["Wv"].T],
                                axis=1).copy(),
            bqkv=np.concatenate([b16["bq"], b16["bk"], b16["bv"]])
            .reshape(1, 3 * P), ones=ones_row))
    r1 = run_bass_kernel_spmd(nc1, in1, core_ids=cores)

    q_shs = []
    k_all = np.zeros((N, P), np.float16)
    v_all = np.zeros((N, P), np.float16)
    for c in cores:
        base, ln = shard_base(c), shard_len(c)
        blob = r1.results[c]["qkv"].reshape(P, NB, 3, P)
        q_shs.append(np.ascontiguousarray(blob[:, :, 0, :].reshape(P, NB * P)))
        k_sh = blob[:, :, 1, :].transpose(1, 0, 2).reshape(NB * P, P)
        v_sh = blob[:, :, 2, :].transpose(1, 0, 2).reshape(NB * P, P)
        k_all[base:base + ln] = k_sh[:ln]
        v_all[base:base + ln] = v_sh[:ln]

    k8, krs = row_quant_int8(k_all)
    v8, vrs = row_quant_int8(v_all)

    # ---------------- L2
    cmap2 = compute_cmap(src)
    plans2 = []
    for c in cores:
        base, ln = shard_base(c), shard_len(c)
        m = (src >= base) & (src < base + ln)
        plans2.append(CorePlan(cmap2, c, src[m], dst[m], eids[m]))

    nc2 = build_l2(cmap2)
    Bmat = head_mask_matrix()
    kf16, koff, kn8, kn16 = chunk_split(cmap2, *L2_SPLIT, tail=True)
    in2 = []
    for c in cores:
        pl = plans2[c]
        nch = cmap2.nch
        oth = pl.slot_other.reshape(nch, P)
        k8_rows = k8[oth[~kf16]]                       # [n8, P, P] int8
        k16_rows = k_all[oth[kf16]]                    # [n16, P, P] f16
        kst8 = k8_rows.transpose(2, 0, 1).reshape(P, -1).copy() \
            if kn8 else np.zeros((P, P), np.int8)
        kst16 = k16_rows.transpose(2, 0, 1).reshape(P, -1).copy() \
            if kn16 else np.zeros((P, P), np.float16)
        valid = (pl.slot_edge >= 0).astype(np.float32)
        scale = np.where(np.repeat(kf16, P), 1.0, krs[pl.slot_other])
        srow_v = (scale * valid * 0.25).astype(np.float16)
        in2.append(dict(
            q_sh=q_shs[c], kst8=kst8, kst16=kst16,
            srow=np.ascontiguousarray(srow_v.reshape(nch, P).T),
            S_st=pl.onehot_stream(False), ST_st=pl.onehot_stream(True),
            Bm=Bmat))
    r2 = run_bass_kernel_spmd(nc2, in2, core_ids=cores)

    exp_edge = np.zeros((E, H), np.float16)
    rec_all = np.zeros((N, H), np.float16)
    for c in cores:
        pl = plans2[c]
        e_flat = r2.results[c]["exp_out"].reshape(P, cmap2.nch, H) \
            .transpose(1, 0, 2).reshape(cmap2.nslots, H)
        real = pl.slot_edge >= 0
        exp_edge[pl.slot_edge[real]] = e_flat[real]
        base, ln = shard_base(c), shard_len(c)
        rec_sh = r2.results[c]["rec_out"].reshape(P, NB, H) \
            .transpose(1, 0, 2).reshape(NB * P, H)
        rec_all[base:base + ln] = rec_sh[:ln]

    # ---------------- L3
    cmap3 = compute_cmap(dst)
    plans3 = []
    for c in cores:
        base, ln = shard_base(c), shard_len(c)
        m = (dst >= base) & (dst < base + ln)
        plans3.append(CorePlan(cmap3, c, dst[m], src[m], eids[m]))

    nc3 = build_l3(cmap3, with_bias=any_bias_out)
    vf16, voff, vn8, vn16 = chunk_split(cmap3, *L3_SPLIT)
    in3 = []
    for c in cores:
        pl = plans3[c]
        nch = cmap3.nch
        oth = pl.slot_other.reshape(nch, P)
        v8_rows = v8[oth[~vf16]]
        v16_rows = v_all[oth[vf16]]
        vst8 = v8_rows.transpose(1, 0, 2).reshape(P, -1).copy() \
            if vn8 else np.zeros((P, P), np.int8)
        vst16 = v16_rows.transpose(1, 0, 2).reshape(P, -1).copy() \
            if vn16 else np.zeros((P, P), np.float16)
        e_slots = np.zeros((cmap3.nslots, H), np.float16)
        real = pl.slot_edge >= 0
        e_slots[real] = exp_edge[pl.slot_edge[real]]
        # combined per-slot scale: softmax denominator recip at the src node
        # times the src v-row int8 scale (1 for f16 chunks, 0 on padding)
        vscale = np.where(np.repeat(vf16, P), 1.0, vrs[pl.slot_other])
        srw_v = (rec_all[pl.slot_other].astype(np.float32) *
                 (vscale * real.astype(np.float32))[:, None]) \
            .astype(np.float16)
        in3.append(dict(
            vst8=vst8, vst16=vst16,
            exp_in=np.ascontiguousarray(
                e_slots.reshape(nch, P, H).transpose(1, 0, 2)
                .reshape(P, nch * H)),
            srw=np.ascontiguousarray(
                srw_v.reshape(nch, P, H).transpose(1, 0, 2)
                .reshape(P, nch * H)),
            TT_st=pl.onehot_stream(True),
            WoT=w16["Wo"].T.copy(),
            bo_r=b16["bo"].reshape(1, P), ones=ones_row))
    r3 = run_bass_kernel_spmd(nc3, in3, core_ids=cores)

    out = np.zeros((N, F), np.float32)
    for c in cores:
        base, ln = shard_base(c), shard_len(c)
        o = r3.results[c]["outT"].reshape(P, NB, P).transpose(1, 2, 0) \
            .reshape(NB * P, P)
        out[base:base + ln] = o[:ln].astype(np.float32)
    return out


# revision 75
# speedup vs baseline: 1.1706x; 1.0580x over previous
"""Trainium2 Bass kernel for nn_EnhancedReflectiveCognitiveGraph (GNN edge-softmax attention).

Math (see reference):
  q/k/v = x @ W{q,k,v}.T + b ; per-edge scores s_e = <q[src_e], k[dest_e]>_head / 4
  softmax over edges sharing src (max-subtraction skipped: scores ~ N(0,1) so
  exp never overflows and the weights are mathematically identical)
  agg[dest] += w_e * v[src_e] ; out = agg @ Wo.T + bo

Device strategy (8 cores, node-range sharding, three SPMD launches).  No
device-side gathers anywhere: the per-edge k/v rows are delivered as
host-prepared per-slot streams read contiguously at full DMA bandwidth
(a dma_gather of 256B rows runs at half bandwidth with one descriptor per
edge, which profiled ~2x slower).

  L1 (proj): each core computes q/k/v (fp16) for its node shard; one fused
      qkv output blob, per-4-block output DMAs pipelined with the matmuls.
  L2 (src phase): core c owns edges with src in its shard, laid out in
      128-edge chunks grouped by 128-node src block.  q rows are expanded
      per-edge on-chip in [feat x slot] orientation via PE matmuls (rhs =
      streamed fp8 one-hots, stationary q block).  k rows arrive per-slot,
      mostly int8 with per-row scales (applied to the 16x smaller reduced
      scores, fused with the 1/sqrt(D) factor); for half the chunk-groups
      the Act engine converts both multiply operands to packed f16 (psum
      copy + int8 upconvert) so their multiply runs in DVE 2x mode, which
      balances Act against the otherwise-saturated DVE.  The per-head dot
      products contract on the PE against a tiny constant block-diagonal
      matrix, so DVE does no reduction.  exp (Act) -> per-src-block segment
      sums via PE matmuls with transposed one-hots -> reciprocal.  Outputs:
      per-edge exp values and per-node reciprocals.
  L3 (dest phase): core c owns edges with dest in its shard.  v rows arrive
      per-slot (int8 + f16 mix as in L2); weights exp * rec[src] * vscale
      are formed on-device from a host-combined per-slot scale stream, the
      weighted rows are scatter-added into per-dest-block agg via PE
      matmuls with one-hots, then the output projection.
  All three launches are software-pipelined with explicit stage skewing
  (expansion/multiply -> score -> exp/segsum one group apart) so no engine
  queue-stalls on a cross-engine dependency; stream tiles are double- or
  triple-buffered and loaded lazily so the DMA engine never runs ahead of
  or behind the compute wavefront.  Output DMAs issue from the otherwise
  idle Pool queue to avoid head-of-line blocking of the input streams.
  Host between launches does relayout only: assembling tables from L1/L2
  outputs, per-row int8 packing, per-slot stream/one-hot construction, and
  permutation of edge exp values between the src- and dest-groupings.
"""

import math
import ml_dtypes
import numpy as np

import concourse.bacc as bacc
import concourse.mybir as mybir
import concourse.tile as tile
from concourse.bass_utils import run_bass_kernel_spmd

# ---------------------------------------------------------------- constants
N = 50000
E = 600000
F = 128
H = 8
Dh = 16
P = 128
C = 8                     # cores
SH = 6272                 # nodes per core, cores 0-6 (49 blocks); core 7: 6096
NB = 49                   # blocks per shard
G = 8                     # chunks per processing group (psum-sized)
KB = 32                   # chunks per stream DMA tile (L2)
KB3 = 64                  # chunks per stream DMA tile (L3)
SG = 12                   # blocks per recip supergroup in L2
F16 = mybir.dt.float16
F8 = mybir.dt.float8e4
F32 = mybir.dt.float32
I8 = mybir.dt.int8


def shard_base(c):
    return c * SH


def shard_len(c):
    return min(N, (c + 1) * SH) - c * SH


# ---------------------------------------------------------------- host prep
class ChunkMap:
    """Uniform chunk structure shared by all cores for one phase.

    Chunks (128 slots each) are block-major: kb[b] chunks for block b; the
    chunk->block map is identical on every core so one program serves all 8."""

    def __init__(self, kb):
        self.kb = [int(x) for x in kb]
        self.chunks = [b for b in range(NB) for _ in range(self.kb[b])]
        self.nch = len(self.chunks)
        self.nslots = self.nch * P
        self.start = np.concatenate([[0], np.cumsum(self.kb)]).astype(int)


def compute_cmap(key, other=None):
    """Global uniform per-block chunk counts for one phase."""
    kb = np.ones(NB, np.int64)
    for c in range(C):
        base, ln = shard_base(c), shard_len(c)
        m = (key >= base) & (key < base + ln)
        cnt = np.bincount((key[m] - base) // P, minlength=NB)
        kb = np.maximum(kb, (cnt + P - 1) // P)
    return ChunkMap(kb)


def group_list(cmap):
    """Processing groups of up to G chunks, block-aligned: (b, g0, gn, cs, ce)."""
    groups = []
    for b in range(NB):
        cs, ce = int(cmap.start[b]), int(cmap.start[b + 1])
        for g0 in range(cs, ce, G):
            groups.append((b, g0, min(G, ce - g0), cs, ce))
    return groups


def chunk_split(cmap, num, den, tail=False, shift=0):
    """Mixed-precision chunk routing: `num` of every `den` groups take the
    f16 path (Act-assisted 2x multiply), the rest the int8 path (half DMA).
    `tail` places the f16 groups at the end of each cycle (keeps the large
    f16 stream tiles off the pipeline ramp).
    Returns (f16 flag per chunk, compact offset per chunk, n8, n16)."""
    f16 = np.zeros(cmap.nch, bool)
    for i, (b, g0, gn, cs, ce) in enumerate(group_list(cmap)):
        sel = (i % den >= den - num) if tail else ((i - shift) % den < num)
        if sel:
            f16[g0:g0 + gn] = True
    off = np.zeros(cmap.nch, np.int64)
    n8 = n16 = 0
    for ci in range(cmap.nch):
        if f16[ci]:
            off[ci] = n16
            n16 += 1
        else:
            off[ci] = n8
            n8 += 1
    return f16, off, n8, n16


class CorePlan:
    """Per-core slot contents for one phase.  `key` = node defining the block
    (src for L2, dest for L3); `other` = node whose row the slot consumes."""

    def __init__(self, cmap, core, key, other, edge_ids):
        base = shard_base(core)
        nsl = cmap.nslots
        self.slot_local = np.full(nsl, -1, np.int64)
        self.slot_other = np.zeros(nsl, np.int64)
        self.slot_edge = np.full(nsl, -1, np.int64)
        block = (key - base) // P
        for b in range(NB):
            m = block == b
            cnt = int(m.sum())
            if cnt == 0:
                continue
            assert cnt <= cmap.kb[b] * P
            s0 = int(cmap.start[b]) * P
            self.slot_local[s0:s0 + cnt] = key[m] - base - b * P
            self.slot_other[s0:s0 + cnt] = other[m]
            self.slot_edge[s0:s0 + cnt] = edge_ids[m]
        self.cmap = cmap

    def onehot_stream(self, transposed):
        """[128, nch*128] fp8; chunk c at cols c*128:(c+1)*128.
        transposed=False: S [loc, (c,slot)] ; True: ST/TT [slot, (c,loc)].
        Dummy slots are all-zero columns/rows."""
        cm = self.cmap
        out = np.zeros((P, cm.nch * P), dtype=ml_dtypes.float8_e4m3)
        loc = self.slot_local
        sl_all = np.arange(cm.nslots)
        valid = loc >= 0
        ch = sl_all // P
        row = sl_all % P
        if transposed:
            out[row[valid], ch[valid] * P + loc[valid]] = 1.0
        else:
            out[loc[valid], ch[valid] * P + row[valid]] = 1.0
        return out


def row_quant_int8(a16):
    """Per-row symmetric int8 quantization.  Returns (int8 vals, f32 scales)."""
    a = np.asarray(a16, np.float32)
    am = np.abs(a).max(axis=1)
    s = np.where(am > 0, am / 127.0, 1.0).astype(np.float32)
    q = np.clip(np.round(a / s[:, None]), -127, 127).astype(np.int8)
    return q, s


def head_mask_matrix():
    """[128, 8] fp8 block-diagonal ones: B[f, h] = (f // 16 == h)."""
    B = np.zeros((P, H), dtype=ml_dtypes.float8_e4m3)
    for h in range(H):
        B[h * Dh:(h + 1) * Dh, h] = 1.0
    return B


# ---------------------------------------------------------------- L1: projections
def build_l1(with_bias=False):
    nc = bacc.Bacc("TRN2", target_bir_lowering=False, num_devices=C)
    xT = nc.dram_tensor("xT", [P, NB * P], F16, kind="ExternalInput")
    wqkv = nc.dram_tensor("wqkv", [P, 3 * P], F16, kind="ExternalInput")
    bqkv = nc.dram_tensor("bqkv", [1, 3 * P], F16, kind="ExternalInput")
    ones = nc.dram_tensor("ones", [1, P], F16, kind="ExternalInput")
    qkv = nc.dram_tensor("qkv", [P, NB * 3 * P], F16, kind="ExternalOutput")

    with tile.TileContext(nc) as tc:
        with tc.tile_pool(name="const", bufs=1) as cpool, \
             tc.tile_pool(name="stage", bufs=3) as spool, \
             tc.tile_pool(name="psum", bufs=4, space="PSUM") as ppool:
            w_sb = cpool.tile([P, 3 * P], F16, tag="w", name="w_sb")
            nc.sync.dma_start(w_sb[:], wqkv[:])
            b_sb = ones_sb = None
            if with_bias:
                b_sb = cpool.tile([1, 3 * P], F16, tag="b", name="b_sb")
                nc.sync.dma_start(b_sb[:], bqkv[:])
                ones_sb = cpool.tile([1, P], F16, tag="ones", name="ones_sb")
                nc.sync.dma_start(ones_sb[:], ones[:])
            xt = cpool.tile([P, NB * P], F16, tag="xT", name="xt")
            # small first slice so the first matmul starts early
            cuts = [0, 4, 12, 24, 36, NB]
            for a, b in zip(cuts[:-1], cuts[1:]):
                nc.sync.dma_start(xt[:, a * P:b * P], xT[:, a * P:b * P])
            DB = 4   # blocks per output DMA
            osb = cpool.tile([P, NB * 3 * P], F16, tag="osb", name="osb")
            for b in range(NB):
                ps = ppool.tile([P, 3 * P], F32, tag="proj", name="ps")
                if with_bias:
                    nc.tensor.matmul(ps[:], lhsT=xt[:, b * P:(b + 1) * P],
                                     rhs=w_sb[:], start=True, stop=False)
                    nc.tensor.matmul(ps[:], lhsT=ones_sb[:], rhs=b_sb[:],
                                     start=False, stop=True)
                else:
                    nc.tensor.matmul(ps[:], lhsT=xt[:, b * P:(b + 1) * P],
                                     rhs=w_sb[:], start=True, stop=True)
                dstsl = osb[:, b * 3 * P:(b + 1) * 3 * P]
                if b % 2 == 0:
                    nc.vector.tensor_copy(dstsl, ps[:])
                else:
                    nc.scalar.copy(dstsl, ps[:])
                if b % DB == DB - 1 or b == NB - 1:
                    a0 = (b // DB) * DB * 3 * P
                    a1 = (b + 1) * 3 * P
                    nc.gpsimd.dma_start(qkv[:, a0:a1], osb[:, a0:a1])
    nc.compile()
    return nc


# ---------------------------------------------------------------- L2: src phase
L2_SPLIT = (1, 2)   # 2/5 of groups take the f16 k path


def build_l2(cmap):
    nch = cmap.nch
    kf16, koff, n8, n16 = chunk_split(cmap, *L2_SPLIT, tail=True)
    nc = bacc.Bacc("TRN2", target_bir_lowering=False, num_devices=C)
    q_sh = nc.dram_tensor("q_sh", [P, NB * P], F16, kind="ExternalInput")
    kst8 = nc.dram_tensor("kst8", [P, max(n8, 1) * P], I8, kind="ExternalInput")
    kst16 = nc.dram_tensor("kst16", [P, max(n16, 1) * P], F16,
                           kind="ExternalInput")
    srow = nc.dram_tensor("srow", [P, nch], F16, kind="ExternalInput")
    S_st = nc.dram_tensor("S_st", [P, nch * P], F8, kind="ExternalInput")
    ST_st = nc.dram_tensor("ST_st", [P, nch * P], F8, kind="ExternalInput")
    Bm = nc.dram_tensor("Bm", [P, H], F8, kind="ExternalInput")
    exp_out = nc.dram_tensor("exp_out", [P, nch * H], F16, kind="ExternalOutput")
    rec_out = nc.dram_tensor("rec_out", [P, NB * H], F16, kind="ExternalOutput")

    groups = group_list(cmap)
    ng = len(groups)

    with tile.TileContext(nc) as tc:
        with tc.tile_pool(name="res", bufs=1) as rpool, \
             tc.tile_pool(name="kst", bufs=3) as kpool, \
             tc.tile_pool(name="st", bufs=3) as tpool, \
             tc.tile_pool(name="sst", bufs=3) as spool, \
             tc.tile_pool(name="work", bufs=8) as wpool, \
             tc.tile_pool(name="qpsum", bufs=3, space="PSUM") as qpsum, \
             tc.tile_pool(name="bpsum", bufs=2, space="PSUM") as bpsum:
            B_sb = rpool.tile([P, H], F8, tag="B", name="B_sb")
            srow_sb = rpool.tile([P, nch], F16, tag="srow", name="srow_sb")
            late = set()

            def want_late(which, tile_, dram):
                if which not in late:
                    late.add(which)
                    nc.sync.dma_start(tile_[:], dram[:])
            q_sb = rpool.tile([P, NB * P], F16, tag="q", name="q_sb")
            q_spans = set()

            def want_q(b):
                i = b * 4 // NB
                if i not in q_spans:
                    q_spans.add(i)
                    a = (i * NB + 3) // 4 * P
                    bb = ((i + 1) * NB + 3) // 4 * P
                    nc.sync.dma_start(q_sb[:, a:bb], q_sh[:, a:bb])
            exp_sb = rpool.tile([P, nch * H], F16, tag="exp", name="exp_sb")
            seg_sb = rpool.tile([P, NB * H], F32, tag="seg", name="seg_sb")
            rec_sb = rpool.tile([P, NB * H], F16, tag="rec", name="rec_sb")

            k_tiles = {}
            t_tiles = {}
            s_tiles = {}

            def stream(tiles, pool, dram, ci, width=P, dt=I8, total=None,
                       off=None):
                cc = ci if off is None else int(off[ci])
                tot = nch if total is None else total
                t0 = cc // KB * KB
                if t0 not in tiles:
                    t = pool.tile([P, KB * width], dt, tag=dram.name,
                                  name=f"strm_{dram.name}_{t0}")
                    n = min(KB, tot - t0) * width
                    h = G * width if t0 == 0 and n > G * width else 0
                    if h:
                        nc.sync.dma_start(t[:, :h], dram[:, :h])
                    nc.sync.dma_start(t[:, h:n],
                                      dram[:, t0 * width + h:t0 * width + n])
                    tiles[t0] = t
                return tiles[t0], t0, cc

            # Software-pipelined stages, skewed so PE never queue-stalls on a
            # cross-engine dependency:
            #   A(i):   q expansion (PE) + qk multiply (DVE)
            #   B(i-1): score matmuls (PE)
            #   C(i-2): on last group of a block: dequant (DVE), exp (Act),
            #           segment-sum matmuls (PE), seg copy (Act)
            qkTs = {}
            scps = {}

            def prefetch(i):
                # touch the stream spans a group ahead so span-boundary DMA
                # latency never stalls the compute pipeline
                b, g0, gn, cs, ce = groups[i]
                ce2 = min(g0 + gn + G, nch) - 1
                stream(s_tiles, spool, S_st, ce2, dt=F8)
                stream(k_tiles, kpool, kst8, ce2, dt=I8)
                stream(t_tiles, tpool, ST_st, ce2, dt=F8)

            def stage_a(i):
                b, g0, gn, cs, ce = groups[i]
                want_q(b)
                want_q(min(b + 3, NB - 1))
                qeT = qpsum.tile([P, G * P], F32, tag="qeT", name="qeT")
                ci = g0
                while ci < g0 + gn:
                    st, t0, _ = stream(s_tiles, spool, S_st, ci, dt=F8)
                    # pieces must not cross 512-col psum bank boundaries
                    cj = min(g0 + gn, t0 + KB, g0 + ((ci - g0) // 4 + 1) * 4)
                    nsl = (cj - ci) * P
                    nc.tensor.matmul(
                        qeT[:, (ci - g0) * P:(ci - g0) * P + nsl],
                        lhsT=q_sb[:, b * P:(b + 1) * P],
                        rhs=st[:, (ci - t0) * P:(ci - t0) * P + nsl],
                        start=True, stop=True)
                    ci = cj
                qkT = wpool.tile([P, G * P], F16, tag="qkT", name="qkT")
                f16p = bool(kf16[g0])
                in0 = qeT
                if f16p:
                    # Act converts both sides to packed f16 so the multiply
                    # runs in DVE 2x mode (engine balancing, no extra DMA)
                    qe16 = wpool.tile([P, G * P], F16, tag="qe16", name="qe16")
                    nc.scalar.copy(qe16[:, :gn * P], qeT[:, :gn * P])
                    in0 = qe16
                ci = g0
                while ci < g0 + gn:
                    kt, t0, cc = stream(k_tiles, kpool, kst8, ci, dt=I8)
                    cj = min(g0 + gn, ci + (t0 + KB - cc))
                    nsl = (cj - ci) * P
                    in1 = kt[:, (cc - t0) * P:(cc - t0) * P + nsl]
                    if f16p:
                        k16 = wpool.tile([P, G * P], F16, tag="k16",
                                         name="k16")
                        nc.scalar.copy(k16[:, :nsl], in1)
                        in1 = k16[:, :nsl]
                    nc.vector.tensor_mul(
                        qkT[:, (ci - g0) * P:(ci - g0) * P + nsl],
                        in0[:, (ci - g0) * P:(ci - g0) * P + nsl],
                        in1)
                    ci = cj
                qkTs[i] = qkT

            def stage_b(i):
                b, g0, gn, cs, ce = groups[i]
                want_late("B", B_sb, Bm)
                want_late("srow", srow_sb, srow)
                qkT = qkTs.pop(i)
                if g0 == cs:
                    scps[b] = bpsum.tile([P, 17 * H], F32, tag="blk",
                                         name="blkps")
                scp = scps[b]
                for ci in range(g0, g0 + gn):
                    nc.tensor.matmul(
                        scp[:, (ci - cs) * H:(ci - cs + 1) * H],
                        lhsT=qkT[:, (ci - g0) * P:(ci - g0 + 1) * P],
                        rhs=B_sb[:], start=True, stop=True)

            def stage_c(i):
                b, g0, gn, cs, ce = groups[i]
                if g0 + gn != ce:
                    return
                nb = ce - cs
                blkps = scps.pop(b)
                scp = blkps
                sc16 = wpool.tile([P, 16 * H], F16, tag="sc16", name="sc16")
                nc.vector.tensor_mul(
                    sc16[:, :nb * H].rearrange("p (c h) -> p c h", h=H),
                    scp[:, :nb * H].rearrange("p (c h) -> p c h", h=H),
                    srow_sb[:, cs:ce][:, :, None]
                    .broadcast_to([P, nb, H]))
                nc.scalar.activation(
                    out=exp_sb[:, cs * H:ce * H],
                    in_=sc16[:, :nb * H],
                    func=mybir.ActivationFunctionType.Exp,
                    scale=1.0)
                segp = blkps[:, 16 * H:17 * H]
                for ci in range(cs, ce):
                    tt, t0, _ = stream(t_tiles, tpool, ST_st, ci, dt=F8)
                    nc.tensor.matmul(
                        segp,
                        lhsT=tt[:, (ci - t0) * P:(ci - t0 + 1) * P],
                        rhs=exp_sb[:, ci * H:(ci + 1) * H],
                        start=(ci == cs), stop=(ci == ce - 1))
                nc.scalar.copy(seg_sb[:, b * H:(b + 1) * H], segp)
                if b == 23:
                    # first-half reciprocal overlapped with the main pipe
                    seg1a = wpool.tile([P, 24 * H], F32, tag="seg1",
                                       name="seg1a")
                    nc.vector.scalar_tensor_tensor(
                        out=seg1a[:], in0=seg_sb[:, :24 * H], scalar=0.0,
                        in1=seg_sb[:, :24 * H],
                        op0=mybir.AluOpType.is_le, op1=mybir.AluOpType.add)
                    with nc.allow_low_precision(reason="recip fits f16"):
                        nc.vector.reciprocal(rec_sb[:, :24 * H], seg1a[:])

            for i in range(ng + 2):
                if i < ng:
                    stage_a(i)
                    prefetch(i)
                if 1 <= i <= ng:
                    stage_b(i - 1)
                if 2 <= i <= ng + 1:
                    stage_c(i - 2)
                    bdone = groups[i - 2][0]
                    if groups[i - 2][1] + groups[i - 2][2] == groups[i - 2][4]:
                        # exp_out slice per ~8 finished blocks
                        if bdone % 8 == 7 or bdone == NB - 1:
                            a = int(cmap.start[bdone // 8 * 8]) * H
                            bcol = int(cmap.start[bdone + 1]) * H
                            nc.gpsimd.dma_start(exp_out[:, a:bcol],
                                                exp_sb[:, a:bcol])

            # reciprocal; empty segments (zero-degree locs, padding) get
            # seg+1 so it stays finite — their one-hot columns are all-zero
            # downstream so the value never contributes.
            seg1 = wpool.tile([P, NB * H], F32, tag="seg1", name="seg1")
            nc.vector.scalar_tensor_tensor(
                out=seg1[:, :(NB - 24) * H], in0=seg_sb[:, 24 * H:],
                scalar=0.0, in1=seg_sb[:, 24 * H:],
                op0=mybir.AluOpType.is_le, op1=mybir.AluOpType.add)
            with nc.allow_low_precision(reason="softmax recip fits f16"):
                nc.vector.reciprocal(rec_sb[:, 24 * H:],
                                     seg1[:, :(NB - 24) * H])
            nc.gpsimd.dma_start(rec_out[:], rec_sb[:])
    nc.compile()
    return nc


# ---------------------------------------------------------------- L3: dest phase
L3_SPLIT = (2, 5)   # 2/5 of groups take the f16 v path


def build_l3(cmap, with_bias=False):
    nch = cmap.nch
    vf16, voff, n8, n16 = chunk_split(cmap, *L3_SPLIT)
    nc = bacc.Bacc("TRN2", target_bir_lowering=False, num_devices=C)
    vst8 = nc.dram_tensor("vst8", [P, max(n8, 1) * P], I8, kind="ExternalInput")
    vst16 = nc.dram_tensor("vst16", [P, max(n16, 1) * P], F16,
                           kind="ExternalInput")
    wsrw = nc.dram_tensor("wsrw", [P, 2 * nch * H], F16, kind="ExternalInput")
    TT_st = nc.dram_tensor("TT_st", [P, nch * P], F8, kind="ExternalInput")
    WoT = nc.dram_tensor("WoT", [P, P], F16, kind="ExternalInput")
    bo_r = nc.dram_tensor("bo_r", [1, P], F16, kind="ExternalInput")
    ones = nc.dram_tensor("ones", [1, P], F16, kind="ExternalInput")
    outT = nc.dram_tensor("outT", [P, NB * P], F16, kind="ExternalOutput")

    with tile.TileContext(nc) as tc:
        with tc.tile_pool(name="res", bufs=1) as rpool, \
             tc.tile_pool(name="vstp", bufs=2) as vpool, \
             tc.tile_pool(name="tt", bufs=2) as tpool, \
             tc.tile_pool(name="work", bufs=3) as wpool, \
             tc.tile_pool(name="apsum", bufs=2, space="PSUM") as apsum, \
             tc.tile_pool(name="opsum", bufs=2, space="PSUM") as opsum:
            wo_sb = rpool.tile([P, P], F16, tag="wo", name="wo_sb")
            nc.sync.dma_start(wo_sb[:], WoT[:])
            bo_sb = ones_sb = None
            if with_bias:
                bo_sb = rpool.tile([1, P], F16, tag="bo", name="bo_sb")
                nc.sync.dma_start(bo_sb[:], bo_r[:])
                ones_sb = rpool.tile([1, P], F16, tag="ones", name="ones_sb")
                nc.sync.dma_start(ones_sb[:], ones[:])
            osb = rpool.tile([P, NB * P], F16, tag="osb", name="osb")
            ws_sb = rpool.tile([P, 2 * nch * H], F16, tag="ws", name="ws_sb")
            wsc = rpool.tile([P, nch * H], F16, tag="wsc", name="wsc")

            v_tiles = {}
            v16_tiles = {}
            t_tiles = {}
            w_spans = set()

            def stream(tiles, pool, dram, ci, dt, total=None, off=None,
                       span=KB3):
                cc = ci if off is None else int(off[ci])
                tot = nch if total is None else total
                t0 = cc // span * span
                if t0 not in tiles:
                    t = pool.tile([P, span * P], dt, tag=dram.name,
                                  name=f"strm_{dram.name}_{t0}")
                    n = min(span, tot - t0) * P
                    h = G * P if t0 == 0 and n > G * P else 0
                    if h:
                        nc.sync.dma_start(t[:, :h], dram[:, :h])
                    nc.sync.dma_start(t[:, h:n],
                                      dram[:, t0 * P + h:t0 * P + n])
                    tiles[t0] = t
                return tiles[t0], t0, cc

            def want_wsc(ci):
                # per-edge weight exp * (rec[src] * v-row-scale): lazily DMA'd
                # and computed (packed f16 2x mult) per KB3-chunk span
                t0 = ci // KB3 * KB3
                if t0 not in w_spans:
                    w_spans.add(t0)
                    a = t0 * H
                    bcol = min(nch, t0 + KB3) * H
                    nc.sync.dma_start(ws_sb[:, 2 * a:2 * bcol],
                                      wsrw[:, 2 * a:2 * bcol])
                    nc.vector.tensor_mul(
                        wsc[:, a:bcol],
                        ws_sb[:, 2 * a:2 * a + (bcol - a)],
                        ws_sb[:, 2 * a + (bcol - a):2 * bcol])

            DB = 7
            groups = group_list(cmap)
            ng = len(groups)
            wvs = {}
            aggps = {}

            def stage_v(i):
                b, g0, gn, cs, ce = groups[i]
                wv = wpool.tile([P, G * P], F16, tag="wv", name="wv")
                want_wsc(g0)
                want_wsc(g0 + gn - 1)
                f16p = bool(vf16[g0])
                erep = None
                if f16p:
                    # materialize the broadcast weights on Act so the
                    # multiply runs packed-f16 at DVE 2x
                    erep = wpool.tile([P, G * P], F16, tag="erep",
                                      name="erep")
                    nc.scalar.copy(
                        erep[:, :gn * P]
                        .rearrange("p (c h d) -> p c h d", h=H, d=Dh),
                        wsc[:, g0 * H:(g0 + gn) * H]
                        .rearrange("p (c h) -> p c h", h=H)[:, :, :, None]
                        .broadcast_to([P, gn, H, Dh]))
                ci = g0
                while ci < g0 + gn:
                    if f16p:
                        vt, t0, cc = stream(v16_tiles, vpool, vst16, ci,
                                            F16, total=n16, off=voff,
                                            span=KB3 // 2)
                        cj = min(g0 + gn, ci + (t0 + KB3 // 2 - cc))
                    else:
                        vt, t0, cc = stream(v_tiles, vpool, vst8, ci,
                                            I8, total=n8, off=voff)
                        cj = min(g0 + gn, ci + (t0 + KB3 - cc))
                    nn = cj - ci
                    if f16p:
                        nc.vector.tensor_mul(
                            wv[:, (ci - g0) * P:(ci - g0 + nn) * P],
                            vt[:, (cc - t0) * P:(cc - t0 + nn) * P],
                            erep[:, (ci - g0) * P:(ci - g0 + nn) * P])
                    else:
                        nc.vector.tensor_mul(
                            wv[:, (ci - g0) * P:(ci - g0 + nn) * P]
                            .rearrange("p (c h d) -> p c h d", h=H, d=Dh),
                            vt[:, (cc - t0) * P:(cc - t0 + nn) * P]
                            .rearrange("p (c h d) -> p c h d", h=H, d=Dh),
                            wsc[:, ci * H:(ci + nn) * H]
                            .rearrange("p (c h) -> p c h", h=H)
                            [:, :, :, None]
                            .broadcast_to([P, nn, H, Dh]))
                    ci = cj
                wvs[i] = wv

            def stage_m(i):
                b, g0, gn, cs, ce = groups[i]
                wv = wvs.pop(i)
                if g0 == cs:
                    aggps[b] = apsum.tile([P, P], F32, tag="agg", name="aggp")
                aggp = aggps[b]
                for ci in range(g0, g0 + gn):
                    tt, t0, _ = stream(t_tiles, tpool, TT_st, ci, F8)
                    nc.tensor.matmul(
                        aggp[:],
                        lhsT=wv[:, (ci - g0) * P:(ci - g0 + 1) * P],
                        rhs=tt[:, (ci - t0) * P:(ci - t0 + 1) * P],
                        start=(ci == cs), stop=(ci == ce - 1))
                if g0 + gn != ce:
                    return
                aggp = aggps.pop(b)
                agg16 = wpool.tile([P, P], F16, tag="agg16", name="agg16")
                nc.scalar.copy(agg16[:], aggp[:])
                outp = opsum.tile([P, P], F32, tag="outp", name="outp")
                if with_bias:
                    nc.tensor.matmul(outp[:], lhsT=wo_sb[:], rhs=agg16[:],
                                     start=True, stop=False)
                    nc.tensor.matmul(outp[:], lhsT=bo_sb[:], rhs=ones_sb[:],
                                     start=False, stop=True)
                else:
                    nc.tensor.matmul(outp[:], lhsT=wo_sb[:], rhs=agg16[:],
                                     start=True, stop=True)
                nc.scalar.copy(osb[:, b * P:(b + 1) * P], outp[:])
                if b % DB == DB - 1 or b == NB - 1:
                    a0 = (b // DB) * DB * P
                    a1 = (b + 1) * P
                    nc.gpsimd.dma_start(outT[:, a0:a1], osb[:, a0:a1])

            for i in range(ng + 1):
                if i < ng:
                    stage_v(i)
                if 1 <= i <= ng:
                    stage_m(i - 1)
    nc.compile()
    return nc


# ---------------------------------------------------------------- orchestration
def _prep_weights(Wq, bq, Wk, bk, Wv, bv, Wo, bo):
    w16 = {k: np.asarray(v, np.float32).astype(np.float16)
           for k, v in (("Wq", Wq), ("Wk", Wk), ("Wv", Wv), ("Wo", Wo))}
    b16 = {k: np.asarray(v, np.float32).astype(np.float16)
           for k, v in (("bq", bq), ("bk", bk), ("bv", bv), ("bo", bo))}
    return w16, b16


def kernel(node_features, edge_index, Wq, bq, Wk, bk, Wv, bv, Wo, bo):
    node_features = np.asarray(node_features, np.float32)
    edge_index = np.asarray(edge_index)
    src, dst = edge_index[0].astype(np.int64), edge_index[1].astype(np.int64)
    x16 = node_features.astype(np.float16)
    w16, b16 = _prep_weights(Wq, bq, Wk, bk, Wv, bv, Wo, bo)
    ones_row = np.ones((1, P), np.float16)
    cores = list(range(C))
    eids = np.arange(E, dtype=np.int64)

    any_bias_in = any(np.any(np.asarray(b) != 0) for b in (bq, bk, bv))
    any_bias_out = bool(np.any(np.asarray(bo) != 0))

    # ---------------- L1
    nc1 = build_l1(with_bias=any_bias_in)
    in1 = []
    for c in cores:
        base, ln = shard_base(c), shard_len(c)
        xt = np.zeros((P, NB * P), np.float16)
        xt[:, :ln] = x16[base:base + ln].T
        in1.append(dict(
            xT=xt,
            wqkv=np.concatenate([w16["Wq"].T, w16["Wk"].T, w16["Wv"].T],
                                axis=1).copy(),
            bqkv=np.concatenate([b16["bq"], b16["bk"], b16["bv"]])
            .reshape(1, 3 * P), ones=ones_row))
    r1 = run_bass_kernel_spmd(nc1, in1, core_ids=cores)

    q_shs = []
    k_all = np.zeros((N, P), np.float16)
    v_all = np.zeros((N, P), np.float16)
    for c in cores:
        base, ln = shard_base(c), shard_len(c)
        blob = r1.results[c]["qkv"].reshape(P, NB, 3, P)
        q_shs.append(np.ascontiguousarray(blob[:, :, 0, :].reshape(P, NB * P)))
        k_sh = blob[:, :, 1, :].transpose(1, 0, 2).reshape(NB * P, P)
        v_sh = blob[:, :, 2, :].transpose(1, 0, 2).reshape(NB * P, P)
        k_all[base:base + ln] = k_sh[:ln]
        v_all[base:base + ln] = v_sh[:ln]

    k8, krs = row_quant_int8(k_all)
    v8, vrs = row_quant_int8(v_all)

    # ---------------- L2
    cmap2 = compute_cmap(src)
    plans2 = []
    for c in cores:
        base, ln = shard_base(c), shard_len(c)
        m = (src >= base) & (src < base + ln)
        plans2.append(CorePlan(cmap2, c, src[m], dst[m], eids[m]))

    nc2 = build_l2(cmap2)
    Bmat = head_mask_matrix()
    in2 = []
    for c in cores:
        pl = plans2[c]
        nch = cmap2.nch
        kst = k8[pl.slot_other].reshape(nch, P, P).transpose(2, 0, 1) \
            .reshape(P, nch * P).copy()
        valid = (pl.slot_edge >= 0).astype(np.float32)
        srow_v = (krs[pl.slot_other] * valid * 0.25).astype(np.float16)
        in2.append(dict(
            q_sh=q_shs[c], kst8=kst,
            srow=np.ascontiguousarray(srow_v.reshape(nch, P).T),
            S_st=pl.onehot_stream(False), ST_st=pl.onehot_stream(True),
            Bm=Bmat))
    r2 = run_bass_kernel_spmd(nc2, in2, core_ids=cores)

    exp_edge = np.zeros((E, H), np.float16)
    rec_all = np.zeros((N, H), np.float16)
    for c in cores:
        pl = plans2[c]
        e_flat = r2.results[c]["exp_out"].reshape(P, cmap2.nch, H) \
            .transpose(1, 0, 2).reshape(cmap2.nslots, H)
        real = pl.slot_edge >= 0
        exp_edge[pl.slot_edge[real]] = e_flat[real]
        base, ln = shard_base(c), shard_len(c)
        rec_sh = r2.results[c]["rec_out"].reshape(P, NB, H) \
            .transpose(1, 0, 2).reshape(NB * P, H)
        rec_all[base:base + ln] = rec_sh[:ln]

    # ---------------- L3
    cmap3 = compute_cmap(dst)
    plans3 = []
    for c in cores:
        base, ln = shard_base(c), shard_len(c)
        m = (dst >= base) & (dst < base + ln)
        plans3.append(CorePlan(cmap3, c, dst[m], src[m], eids[m]))

    nc3 = build_l3(cmap3, with_bias=any_bias_out)
    vf16, voff, vn8, vn16 = chunk_split(cmap3, *L3_SPLIT)
    in3 = []
    for c in cores:
        pl = plans3[c]
        nch = cmap3.nch
        oth = pl.slot_other.reshape(nch, P)
        v8_rows = v8[oth[~vf16]]
        v16_rows = v_all[oth[vf16]]
        vst8 = v8_rows.transpose(1, 0, 2).reshape(P, -1).copy() \
            if vn8 else np.zeros((P, P), np.int8)
        vst16 = v16_rows.transpose(1, 0, 2).reshape(P, -1).copy() \
            if vn16 else np.zeros((P, P), np.float16)
        e_slots = np.zeros((cmap3.nslots, H), np.float16)
        real = pl.slot_edge >= 0
        e_slots[real] = exp_edge[pl.slot_edge[real]]
        # combined per-slot scale: softmax denominator recip at the src node
        # times the src v-row int8 scale (1 for f16 chunks, 0 on padding)
        vscale = np.where(np.repeat(vf16, P), 1.0, vrs[pl.slot_other])
        srw_v = (rec_all[pl.slot_other].astype(np.float32) *
                 (vscale * real.astype(np.float32))[:, None]) \
            .astype(np.float16)
        w_in = e_slots.reshape(nch, P, H).transpose(1, 0, 2) \
            .reshape(P, nch * H)
        sv_in = srw_v.reshape(nch, P, H).transpose(1, 0, 2) \
            .reshape(P, nch * H)
        wsrw = np.zeros((P, 2 * nch * H), np.float16)
        for t0 in range(0, nch, KB3):
            a, bcol = t0 * H, min(nch, t0 + KB3) * H
            wsrw[:, 2 * a:2 * a + (bcol - a)] = w_in[:, a:bcol]
            wsrw[:, 2 * a + (bcol - a):2 * bcol] = sv_in[:, a:bcol]
        in3.append(dict(
            vst8=vst8, vst16=vst16, wsrw=wsrw,
            TT_st=pl.onehot_stream(True),
            WoT=w16["Wo"].T.copy(),
            bo_r=b16["bo"].reshape(1, P), ones=ones_row))
    r3 = run_bass_kernel_spmd(nc3, in3, core_ids=cores)

    out = np.zeros((N, F), np.float32)
    for c in cores:
        base, ln = shard_base(c), shard_len(c)
        o = r3.results[c]["outT"].reshape(P, NB, P).transpose(1, 2, 0) \
            .reshape(NB * P, P)
        out[base:base + ln] = o[:ln].astype(np.float32)
    return out
